# revision 53
# baseline (speedup 1.0000x reference)
"""Trainium2 Bass kernel for causal multi-head attention (nn_Attention_3161095930536).

Model: batch=2, seq=2048, d_model=1024, 16 heads x 64. Reference computes
QKV projections + causal softmax attention + output projection (+ biases).

Sharding over 8 NeuronCores: core = (batch b = core//4) x (head-group
g = core%4, 4 heads each). Each core computes its head-group's attention and
a partial output projection into DRAM; the HOST sums the 4 partials per batch
(and adds the folded output bias), keeping the device critical path free of
collectives.

v3: single software-pipelined stream over the 4 seq chunks (sc). The v2
two-phase design (project everything, then attend) left the PE idle in
bursts during the early attention chunks, dropping the HAM clock gate to
K=4/8 (1.2 GHz) for ~90us of the kernel, and serialized ~30us of input DMA
at the start. Here the projections / pass-1 / V transposes for chunk sc+1
are emitted as *filler* inside the attention stream of chunk sc, so:
 - the PE has dense matmul work end to end (HAM stays at 2.4 GHz),
 - the projection-phase ACT work (bias adds) overlaps the exp stream,
 - the startup residual DMA overlaps stage-0 projections.

Other changes vs v2:
 - exp runs on the UNMASKED diagonal scores (bounded: pass-1's row max is
   over legal keys, so masked entries exceed it by at most the score range
   ~O(10), well inside exp's fp32/bf16 range); the above-diagonal region of
   the bf16 P tile is then zeroed by gpsimd affine_select. This takes the
   DVE mask-add out of the S2->exp chain entirely.
 - all input DMAs are emitted up front, interleaved (weights mo-chunk,
   then the matching residual chunk) so stage 0 can start ~3us in.

Carried over from v2: fp16 activations/weights (bf16 P/V for exp range),
fused -max row in the S2 matmul (row 64 of qT/kT), stride-16 sampled-key
pass-1 row max (underestimates only; softmax shift-invariance makes any
shift exact), V^T computed with stationary weights then PE-transposed into
[k, d] slabs, AV riding 5 blocks behind S2, host-side reduce of the 4
per-batch partials with b_V/b_O folded into one bias row.
"""

import numpy as np

import concourse.bass as bass
import concourse.mybir as mybir
import concourse.tile as tile
from concourse import bacc
from concourse.bass_utils import run_bass_kernel_spmd
from concourse.masks import make_identity

dt = mybir.dt
AF = mybir.ActivationFunctionType
ALU = mybir.AluOpType
AX = mybir.AxisListType

NUM_HEADS = 16
D_MODEL = 1024
D_HEAD = 64
D_SEQ = 2048
BATCH = 2
N_CORES = 8
HPG = 4          # heads per group (per core)
G = 4            # groups per batch
SQ = 512         # seq chunk (pipeline stage)
MO = D_MODEL // 128   # 8 m-chunks
NQT = D_SEQ // 128    # 16 q tiles
NQC = D_SEQ // SQ     # 4 seq chunks / stages
STRIDE = 16           # pass-1 past-key subsample stride
NSAMP = (D_SEQ - 128) // STRIDE   # 120 sampled past keys max
AV_LAG = 5            # A*V rides this many blocks behind S2
# last head of each stage is even -> its attnT write is a direct DVE store
# (no DMA down), shortening the chain into the next stage's outproj
HEAD_ORDER = [1, 3, 0, 2]

_prog_cache = {}


def _build_program():
    nc = bacc.Bacc("TRN2", target_bir_lowering=False, debug=False,
                   num_devices=N_CORES)

    resT_in = nc.dram_tensor("resT", [128, NQC, MO, SQ], dt.float16, kind="ExternalInput").ap()
    wq_in = nc.dram_tensor("wq", [128, MO, 2, 128], dt.float16, kind="ExternalInput").ap()
    wk_in = nc.dram_tensor("wk", [128, MO, 2, 128], dt.float16, kind="ExternalInput").ap()
    wv_in = nc.dram_tensor("wv", [128, MO, 2, 128], dt.float16, kind="ExternalInput").ap()
    bq_in = nc.dram_tensor("bq", [128, 2], dt.float32, kind="ExternalInput").ap()
    bk_in = nc.dram_tensor("bk", [128, 2], dt.float32, kind="ExternalInput").ap()
    wo_in = nc.dram_tensor("wo", [128, 2, D_MODEL], dt.float16, kind="ExternalInput").ap()
    out_io = nc.dram_tensor("out_part", [D_SEQ, D_MODEL], dt.float32, kind="ExternalOutput").ap()

    with tile.TileContext(nc) as tc:
        from contextlib import ExitStack
        outer = ExitStack()
        with outer:
            const = outer.enter_context(tc.tile_pool(name="const", bufs=1))
            qkp = outer.enter_context(tc.tile_pool(name="qkp", bufs=1))
            vp = outer.enter_context(tc.tile_pool(name="vp", bufs=1))
            statp = outer.enter_context(tc.tile_pool(name="statp", bufs=1))
            mxsp = outer.enter_context(tc.tile_pool(name="mxsp", bufs=2))
            scrp = outer.enter_context(tc.tile_pool(name="scrp", bufs=3))
            rcp = outer.enter_context(tc.tile_pool(name="rcp", bufs=4))
            rcbp = outer.enter_context(tc.tile_pool(name="rcbp", bufs=4))
            ttp = outer.enter_context(tc.tile_pool(name="ttp", bufs=4))
            ptp = outer.enter_context(tc.tile_pool(name="ptp", bufs=22))
            atp = outer.enter_context(tc.tile_pool(name="atp", bufs=1))
            osp = outer.enter_context(tc.tile_pool(name="osp", bufs=3))
            rp = outer.enter_context(tc.tile_pool(name="rp", bufs=1))
            wp = outer.enter_context(tc.tile_pool(name="wp", bufs=1))
            qtp = outer.enter_context(tc.tile_pool(name="qtp", bufs=3))
            vtp = outer.enter_context(tc.tile_pool(name="vtp", bufs=4))
            psum = outer.enter_context(tc.tile_pool(name="psum", bufs=3, space="PSUM"))
            projps = outer.enter_context(tc.tile_pool(name="projps", bufs=2, space="PSUM"))
            p1ps = outer.enter_context(tc.tile_pool(name="p1ps", bufs=2, space="PSUM"))
            avps = outer.enter_context(tc.tile_pool(name="avps", bufs=1, space="PSUM"))
            dram = outer.enter_context(tc.tile_pool(name="dram", bufs=1, space="DRAM"))

            # ---- constants ----
            ident = const.tile([128, 128], dt.float32r, name="ident")
            ident_f = const.tile([128, 128], dt.float32, name="ident_f")
            make_identity(nc, ident_f[:])
            nc.vector.tensor_copy(ident[:], ident_f[:])
            ident_b = const.tile([128, 128], dt.bfloat16, name="ident_b")
            nc.vector.tensor_copy(ident_b[:], ident_f[:])

            # pass-1 diag mask (S [q, k] orientation: keep where j <= p)
            trimaskT = const.tile([128, 128 + NSAMP], dt.float32, name="trimaskT")
            nc.gpsimd.memset(trimaskT[:], 0.0)
            nc.gpsimd.affine_select(out=trimaskT[:, 0:128], in_=trimaskT[:, 0:128],
                                    compare_op=ALU.is_ge, fill=-1e30,
                                    base=0, pattern=[[-1, 128]], channel_multiplier=1)

            bqs = const.tile([128, 2], dt.float32, name="bqs")
            bks = const.tile([128, 2], dt.float32, name="bks")
            # issue the first gpsimd DMA and partition_broadcast immediately:
            # each triggers a one-time ~6us IRAM library load on the Q7 cores
            # that must overlap the startup DMA wait, not stall the gpsimd
            # queue mid-kernel
            nc.gpsimd.dma_start(bqs[:], bq_in[:])
            warmb = const.tile([64, 32], dt.float32, name="warmb")
            nc.gpsimd.partition_broadcast(warmb[:], ident_f[0:1, 0:32])
            nc.sync.dma_start(bks[:], bk_in[:])

            # dummy matmuls: keep the PE busy from ~1.5us so the HAM clock
            # gate reaches K=8/8 (2.4 GHz) before the first projection's
            # input DMA lands (~10us); otherwise the whole first projection
            # wave runs at 1.2 GHz
            wps = psum.tile([128, 512], dt.float32, name="warmps", tag="ps")
            for _ in range(36):
                nc.tensor.matmul(wps[:, 0:128], ident_b[:], ident_b[:],
                                 start=True, stop=True)
            warms = const.tile([1, 1], dt.float32, name="warms")
            nc.vector.tensor_copy(warms[:], wps[0:1, 0:1])
            # dummy exp: loads the ACT table set (~2.7us) during startup
            warme = const.tile([1, 1], dt.float32, name="warme")
            nc.scalar.activation(warme[:], ident_f[0:1, 0:1], AF.Exp)


            # ---- persistent activations ----
            qT = [qkp.tile([65, D_SEQ], dt.float16, name=f"qT{h}") for h in range(HPG)]
            kT = [qkp.tile([65, D_SEQ], dt.float16, name=f"kT{h}") for h in range(HPG)]
            kTs = [qkp.tile([64, NSAMP], dt.float16, name=f"kTs{h}") for h in range(HPG)]
            # V in [k, d] layout, one 65-wide slab per head: cols 0:64 = V_h, col 64 = 1.0
            vkd = vp.tile([128, NQT, HPG, 65], dt.bfloat16, name="vkd")

            attnT = atp.tile([128, 2, D_SEQ], dt.float16, name="attnT")
            wo = atp.tile([128, 2, D_MODEL], dt.float16, name="wo")

            # negmx[h][sc] col qt%4 = -(max over sampled+diag keys) per q row
            negmx = [[statp.tile([128, 32], dt.float32r, name=f"negmx{h}_{sc}")
                      for sc in range(NQC)] for h in range(HPG)]

            # ---- weights + residual: few big DMAs (the Sync engine spends
            # ~0.6us of issue time per DMA instruction, so DMA count matters
            # more than transfer size); residual is laid out per-stage
            # contiguous on the host so each stage is one descriptor run ----
            wq = wp.tile([128, MO, 2, 128], dt.float16, name="wq")
            wk = wp.tile([128, MO, 2, 128], dt.float16, name="wk")
            wv = wp.tile([128, MO, 2, 128], dt.float16, name="wv")
            resT = [rp.tile([128, MO, SQ], dt.float16, name=f"resT{sc}")
                    for sc in range(NQC)]
            # inputs split across BOTH hardware DGE queues (SP + Activation)
            # so the ~6.5MB load halves in time; the ACT engine only pays a
            # ~0.6us issue cost per DMA, long before its first real work
            nc.sync.dma_start(wq[:, 0:4], wq_in[:, 0:4])
            nc.scalar.dma_start(wk[:, 0:4], wk_in[:, 0:4])
            nc.sync.dma_start(resT[0][:, 0:4], resT_in[:, 0, 0:4])
            nc.scalar.dma_start(wk[:, 4:8], wk_in[:, 4:8])
            nc.sync.dma_start(wq[:, 4:8], wq_in[:, 4:8])
            nc.sync.dma_start(resT[0][:, 4:8], resT_in[:, 0, 4:8])
            nc.scalar.dma_start(wv[:], wv_in[:])
            nc.scalar.dma_start(wo[:], wo_in[:])
            nc.sync.dma_start(resT[1][:], resT_in[:, 1])
            nc.scalar.dma_start(resT[2][:], resT_in[:, 2])
            nc.sync.dma_start(resT[3][:], resT_in[:, 3])
            # persistent-tile memsets AFTER the warmup emission so the DVE
            # queue doesn't delay the ident casts the warmup matmuls need
            nc.vector.memset(vkd[:, :, :, 64], 1.0)
            for h in range(HPG):
                nc.vector.memset(kT[h][64:65, :], 1.0)

            # ================= stage building blocks =================

            def proj_half(sc, which, p, half):
                """Half a projection group: 4 accumulating matmuls; the
                second half also drains psum -> qT/kT/vT."""
                w, dst, bias = {
                    "q": (wq, qT, bqs), "k": (wk, kT, bks), "v": (wv, None, None),
                }[which]
                if half == 0:
                    ps = projps.tile([128, SQ], dt.float32,
                                     name=f"ps_{which}{p}_{sc}", tag="proj")
                    proj_half.live[(sc, which, p)] = ps
                else:
                    ps = proj_half.live.pop((sc, which, p))
                for mo in range(4 * half, 4 * half + 4):
                    nc.tensor.matmul(ps[:], w[:, mo, p, :], resT[sc][:, mo],
                                     start=(mo == 0), stop=(mo == MO - 1))
                if half == 0:
                    return
                cols = slice(sc * SQ, (sc + 1) * SQ)
                if which == "v":
                    vt = vtp.tile([128, SQ], dt.bfloat16, name=f"vT{p}_{sc}", tag="vt")
                    nc.scalar.copy(vt[:], ps[:])
                    proj_half.vt[(sc, p)] = vt
                    return
                # even head: direct ACT with bias
                nc.scalar.activation(dst[2 * p][0:64, cols], ps[0:64, :], AF.Identity,
                                     bias=bias[0:64, p:p + 1], scale=1.0)
                # odd head: aligned ACT into tmp rows 64:128, then DMA down
                qt_t = qtp.tile([128, SQ], dt.float16, name=f"qtmp_{which}{p}{sc}", tag="qtmp")
                nc.scalar.activation(qt_t[64:128, :], ps[64:128, :], AF.Identity,
                                     bias=bias[64:128, p:p + 1], scale=1.0)
                nc.gpsimd.dma_start(dst[2 * p + 1][0:64, cols], qt_t[64:128, :])
            proj_half.live = {}
            proj_half.vt = {}

            def kts_copy(sc, h):
                # sampled keys (stride 16) newly available from seq chunk sc
                lo = sc * SQ // STRIDE
                hi = min((sc + 1) * SQ, D_SEQ - 128) // STRIDE
                if hi <= lo:
                    return
                nc.vector.tensor_copy(
                    kTs[h][:, lo:hi],
                    kT[h][0:64, sc * SQ:hi * STRIDE:STRIDE])

            def vt_transpose(sc, kc_local):
                kc = 4 * sc + kc_local
                ps = psum.tile([128, 512], dt.float32, name=f"ps_v{kc}", tag="ps")
                pb = ps[:].bitcast(dt.bfloat16)
                for c in range(2):
                    nc.tensor.transpose(pb[:, 512 * c:512 * c + 128],
                                        proj_half.vt[(sc, c)][:, kc_local * 128:(kc_local + 1) * 128],
                                        ident_b[:])
                nc.vector.tensor_copy(
                    vkd[:, kc, :, 0:64].rearrange("p (c h) d -> p c h d", c=2),
                    pb.rearrange("p (c r) -> p c r", c=2)[:, :, 0:128]
                      .rearrange("p c (h d) -> p c h d", h=2))

            def p1_block(h, qt):
                nsamp = (qt * 128) // STRIDE
                ncols = 128 + nsamp
                ps = p1ps.tile([128, 512], dt.float32, name=f"ps_p1_{h}_{qt}", tag="p1")
                qstat = qT[h][0:64, qt * 128:(qt + 1) * 128]
                nc.tensor.matmul(ps[:, 0:128], qstat,
                                 kT[h][0:64, qt * 128:(qt + 1) * 128],
                                 start=True, stop=True)
                if nsamp:
                    nc.tensor.matmul(ps[:, 128:ncols], qstat,
                                     kTs[h][:, 0:nsamp], start=True, stop=True)
                scr = scrp.tile([128, 128 + NSAMP], dt.bfloat16,
                                name=f"scr{h}_{qt}", tag="scr")
                nc.vector.tensor_tensor(scr[:, 0:ncols], ps[:, 0:ncols],
                                        trimaskT[:, 0:ncols], ALU.add)
                nc.vector.tensor_reduce(negmx[h][qt // 4][:, (qt % 4):(qt % 4) + 1],
                                        scr[:, 0:ncols], AX.X, ALU.max, negate=True)

            def p1_final(h, sc):
                # -max column -> row 64 of qT[h] for this chunk (via transpose
                # on the PE and a DRAM partition->row reshape hop)
                pst = psum.tile([128, 512], dt.float32r, name=f"ps_tp{h}_{sc}", tag="ps")
                nc.tensor.transpose(pst[0:32, 0:128], negmx[h][sc][:], ident[:])
                mxs = mxsp.tile([4, 128], dt.float16, name=f"mxs{h}_{sc}", tag="mxs")
                nc.vector.tensor_copy(mxs[:], pst[0:4, 0:128].bitcast(dt.float32))
                mrow = dram.tile([4, 128], dt.float16, name=f"mrow{h}_{sc}")
                nc.sync.dma_start(mrow[:], mxs[:])
                nc.sync.dma_start(qT[h][64:65, sc * SQ:(sc + 1) * SQ],
                                  mrow[:].rearrange("t f -> (t f)").unsqueeze(0))

            def outproj_sub(qc, sub):
                st = 4 * qc + sub
                final = (qc == NQC - 1)
                osb = osp.tile([128, D_MODEL], dt.float32, name=f"osb{st}", tag="osb")
                for mc in range(2):
                    ps = psum.tile([128, 512], dt.float32, name=f"ps_o{st}_{mc}", tag="ps")
                    for eo in range(2):
                        nc.tensor.matmul(ps[:], attnT[:, eo, st * 128:(st + 1) * 128],
                                         wo[:, eo, mc * 512:(mc + 1) * 512],
                                         start=(eo == 0), stop=(eo == 1))
                    # the final chunk drains via ACT (idle after the last exp)
                    # so its psum copies don't serialize with the matmuls
                    if final:
                        nc.scalar.copy(osb[:, mc * 512:(mc + 1) * 512], ps[:])
                    else:
                        nc.vector.tensor_copy(osb[:, mc * 512:(mc + 1) * 512], ps[:])
                # final-chunk output alternates the two HWDGE queues so the
                # last transfers run in parallel instead of serializing
                eng = (nc.sync if sub % 2 == 0 else nc.scalar) if final else nc.gpsimd
                eng.dma_start(out_io[st * 128:(st + 1) * 128, :], osb[:])

            # ---- filler machinery: work for stage sc+1, emitted inside
            # the attention stream of stage sc ----
            def make_filler(sc):
                """Filler items that prepare stage sc (emitted during stage
                sc-1's attention). Pass-1 comes right after the q/k
                projections so its DMA-hop latency hides behind the V
                projections / transposes; outproj has no dependencies on this
                stage and goes last as pure PE filler."""
                items = []
                for which, p in (("q", 0), ("q", 1), ("k", 0), ("k", 1)):
                    for half in range(2):
                        items.append(lambda w=which, p=p, hf=half: proj_half(sc, w, p, hf))
                for h in HEAD_ORDER:
                    items.append(lambda h=h: kts_copy(sc, h))
                # V before pass-1: its DVE psum drain must not queue behind
                # pass-1's DVE adds/reduces (the V transposes need it)
                for p in range(2):
                    for half in range(2):
                        items.append(lambda p=p, hf=half: proj_half(sc, "v", p, hf))
                for kcl in range(4):
                    items.append(lambda kcl=kcl: vt_transpose(sc, kcl))
                for h in HEAD_ORDER:
                    for qt in range(4 * sc, 4 * sc + 4):
                        items.append(lambda h=h, qt=qt: p1_block(h, qt))
                    items.append(lambda h=h: p1_final(h, sc))
                # outproj last: pure PE work hiding the pass-1 DMA-hop latency
                if sc >= 2:
                    for sub in range(4):
                        items.append(lambda sub=sub: outproj_sub(sc - 2, sub))
                return items

            class Filler:
                def __init__(self, items, slots):
                    self.items = items
                    self.per = len(items) / max(1, slots)
                    self.budget = 0.0
                    self.idx = 0

                def step(self, mult=1.0):
                    self.budget += self.per * mult
                    while self.idx < min(len(self.items), int(self.budget)):
                        self.items[self.idx]()
                        self.idx += 1

                def drain(self):
                    while self.idx < len(self.items):
                        self.items[self.idx]()
                        self.idx += 1

            # ================= stage 0 preamble =================
            for which, p in (("q", 0), ("q", 1), ("k", 0), ("k", 1)):
                for half in range(2):
                    proj_half(0, which, p, half)
            for h in HEAD_ORDER:
                kts_copy(0, h)
            for p in range(2):
                for half in range(2):
                    proj_half(0, "v", p, half)
            for kcl in range(4):
                vt_transpose(0, kcl)
            for h in HEAD_ORDER:
                for qt in range(4):
                    p1_block(h, qt)
                p1_final(h, 0)

            # ================= pipelined attention stages =================
            for sc in range(NQC):
                nkt = 4 * sc + 4
                # stage 3 keeps outproj(2) for the exp-throttled gaps of its
                # last head (emitted there directly) instead of generic filler
                fill = Filler(make_filler(sc + 1) if sc + 1 < NQC else [],
                              slots=HPG * (nkt + 1))
                for h in HEAD_ORDER:
                    # emit filler ahead of the first S2 so the PE queue has
                    # work while the S2 waits on cross-engine dependencies
                    fill.step(2.0 if h == HEAD_ORDER[0] else 1.0)
                    pt_blks = {}
                    av = avps.tile([128, 512], dt.float32, name=f"ps_av_{h}_{sc}", tag="av")

                    def emit_av(kt, av=av, pt_blks=pt_blks, sc=sc, h=h, nkt=nkt):
                        # diag blocks: cols < 128r are fully masked (zero in
                        # pt), so skip them — earlier kt blocks own those q's
                        r = max(0, kt - 4 * sc)
                        nc.tensor.matmul(av[0:65, 128 * r:], vkd[:, kt, h, :],
                                         pt_blks[kt][:, 128 * r:],
                                         start=(kt == 0), stop=(kt == nkt - 1),
                                         skip_group_check=True)

                    for kt in range(nkt):
                        ps = psum.tile([128, 512], dt.float32, name=f"ps_s2_{h}_{sc}_{kt}", tag="ps")
                        r = max(0, kt - 4 * sc)
                        nc.tensor.matmul(ps[:, 128 * r:],
                                         kT[h][:, kt * 128:(kt + 1) * 128],
                                         qT[h][:, sc * SQ + 128 * r:(sc + 1) * SQ],
                                         start=True, stop=True)
                        pt = ptp.tile([128, 512], dt.bfloat16, name=f"pt{h}_{sc}_{kt}", tag="pt")
                        nc.scalar.activation(pt[:, 128 * r:], ps[:, 128 * r:], AF.Exp)
                        if kt - 4 * sc >= 0:
                            # zero above-diagonal of the exp'd diag sub-block
                            nc.gpsimd.affine_select(
                                out=pt[:, 128 * r:128 * r + 128],
                                in_=pt[:, 128 * r:128 * r + 128],
                                compare_op=ALU.is_ge, fill=0.0,
                                base=0, pattern=[[1, 128]], channel_multiplier=-1)
                        pt_blks[kt] = pt
                        if kt >= AV_LAG:
                            emit_av(kt - AV_LAG)
                        fill.step()
                        if sc == NQC - 1 and h == HEAD_ORDER[-1] and kt % 4 == 3:
                            outproj_sub(NQC - 2, kt // 4)
                    for kt in range(max(0, nkt - AV_LAG), nkt):
                        emit_av(kt)
                    # normalize: broadcast the Z row to 64 partitions, then a
                    # full-rate approximate reciprocal (51 ULP, plenty for the
                    # softmax denominator) — no DMA round trips in the chain
                    zsb = rcp.tile([65, 512], dt.float32, name=f"zsb{h}_{sc}", tag="zsb", bufs=4)
                    nc.vector.tensor_copy(zsb[:], av[0:65, :])
                    # one row DMA to move Z to partition 0 (partition_broadcast
                    # broadcasts partition 0; engines cannot shift partitions)
                    zrow = rcp.tile([1, 512], dt.float32, name=f"zrow{h}_{sc}", tag="zrow", bufs=4)
                    nc.sync.dma_start(zrow[:], zsb[64:65, :])
                    zbc = rcbp.tile([64, 512], dt.float32, name=f"zbc{h}_{sc}", tag="zbc")
                    nc.gpsimd.partition_broadcast(zbc[:], zrow[:])
                    rcb = rcbp.tile([64, 512], dt.float32, name=f"rcb{h}_{sc}", tag="rcb")
                    nc.vector.reciprocal_approx_fast(rcb[:], zbc[:])
                    eh = h // 2
                    last = (sc == NQC - 1 and h == HEAD_ORDER[-1])
                    if last:
                        # dummy matmuls bridge the PE over the Z-chain latency
                        # so the HAM clock gate stays at 2.4 GHz for the final
                        # output projection
                        wp2 = psum.tile([128, 512], dt.float32, name="warmps2", tag="ps")
                        for _ in range(24):
                            nc.tensor.matmul(wp2[:, 0:128], ident_b[:], ident_b[:],
                                             start=True, stop=True)
                    if h % 2 == 0:
                        if last:
                            # final head of the kernel: normalize in 128-col
                            # pieces, launching each outproj subtile as soon
                            # as its columns land, to shorten the drain tail
                            for st in range(4):
                                cols = slice(sc * SQ + st * 128, sc * SQ + (st + 1) * 128)
                                nc.vector.tensor_tensor(attnT[0:64, eh, cols],
                                                        zsb[0:64, st * 128:(st + 1) * 128],
                                                        rcb[:, st * 128:(st + 1) * 128],
                                                        ALU.mult)
                                outproj_sub(sc, st)
                        else:
                            nc.vector.tensor_tensor(attnT[0:64, eh, sc * SQ:(sc + 1) * SQ],
                                                    zsb[0:64, :], rcb[:], ALU.mult)
                    else:
                        att = ttp.tile([64, 512], dt.float16, name=f"att{h}_{sc}", tag="att")
                        nc.vector.tensor_tensor(att[:], zsb[0:64, :], rcb[:], ALU.mult)
                        nc.gpsimd.dma_start(attnT[64:128, eh, sc * SQ:(sc + 1) * SQ], att[:])
                fill.drain()
            nc.vector.tensor_copy(warms[:], wp2[0:1, 0:1])

    nc.compile()
    return nc


def _get_program():
    if "nc" not in _prog_cache:
        _prog_cache["nc"] = _build_program()
    return _prog_cache["nc"]


def _shard_inputs(residual, W_Q, W_K, W_V, W_O, b_Q, b_K, b_V, b_O):
    f32 = np.float32
    in_maps = []
    for core in range(N_CORES):
        b, g = core // G, core % G
        heads = list(range(HPG * g, HPG * g + HPG))
        # residual^T: [m, s] -> [mi, sc, mo, s-chunk] (per-stage contiguous)
        rT = np.ascontiguousarray(
            residual[b].T.reshape(MO, 128, NQC, SQ).transpose(1, 2, 0, 3)).astype(np.float16)

        def wstack(W, scale=1.0):
            # per pair p: [m, 128] -> [mi, mo, p, 128]
            pairs = []
            for p in range(2):
                wpair = np.concatenate([W[heads[2 * p]], W[heads[2 * p + 1]]], axis=1) * scale
                pairs.append(wpair.reshape(MO, 128, 128).transpose(1, 0, 2))
            return np.ascontiguousarray(np.stack(pairs, axis=2)).astype(np.float16)

        wq = wstack(W_Q, 0.125)
        wk = wstack(W_K)
        wv = wstack(W_V)
        bq = np.stack([np.concatenate([b_Q[heads[2 * p]], b_Q[heads[2 * p + 1]]]) * 0.125
                       for p in range(2)], axis=1).astype(f32)
        bk = np.stack([np.concatenate([b_K[heads[2 * p]], b_K[heads[2 * p + 1]]])
                       for p in range(2)], axis=1).astype(f32)
        wo = np.ascontiguousarray(
            W_O[256 * g:256 * (g + 1)].reshape(2, 128, D_MODEL).transpose(1, 0, 2)
        ).astype(np.float16)
        in_maps.append(dict(resT=rT, wq=wq, wk=wk, wv=wv, bq=np.ascontiguousarray(bq),
                            bk=np.ascontiguousarray(bk), wo=wo))
    return in_maps


def _run(inputs, trace=False):
    nc = _get_program()
    in_maps = _shard_inputs(**inputs)
    res = run_bass_kernel_spmd(nc, in_maps, core_ids=list(range(N_CORES)), trace=trace)
    # host-side reduce over the 4 head-group cores of each batch + folded bias
    W_O, b_V, b_O = inputs["W_O"], inputs["b_V"], inputs["b_O"]
    c = (b_O + sum(b_V[h] @ W_O[64 * h:64 * (h + 1)] for h in range(NUM_HEADS))
         ).astype(np.float32)
    out = np.empty((BATCH, D_SEQ, D_MODEL), np.float32)
    for b in range(BATCH):
        acc = res.results[b * G]["out_part"].astype(np.float32).copy()
        for g in range(1, G):
            acc += res.results[b * G + g]["out_part"]
        out[b] = acc + c[None, :]
    return out, res


def kernel(**inputs):
    out, _ = _run(inputs, trace=False)
    return out


# revision 54
# speedup vs baseline: 1.0411x; 1.0411x over previous
"""Trainium2 Bass kernel for causal multi-head attention (nn_Attention_3161095930536).

Model: batch=2, seq=2048, d_model=1024, 16 heads x 64. Reference computes
QKV projections + causal softmax attention + output projection (+ biases).

Sharding over 8 NeuronCores: core = (batch b = core//4) x (head-group
g = core%4, 4 heads each). Each core computes its head-group's attention and
a partial output projection into DRAM; the HOST sums the 4 partials per batch
(and adds the folded output bias), keeping the device critical path free of
collectives.

v3: single software-pipelined stream over the 4 seq chunks (sc). The v2
two-phase design (project everything, then attend) left the PE idle in
bursts during the early attention chunks, dropping the HAM clock gate to
K=4/8 (1.2 GHz) for ~90us of the kernel, and serialized ~30us of input DMA
at the start. Here the projections / pass-1 / V transposes for chunk sc+1
are emitted as *filler* inside the attention stream of chunk sc, so:
 - the PE has dense matmul work end to end (HAM stays at 2.4 GHz),
 - the projection-phase ACT work (bias adds) overlaps the exp stream,
 - the startup residual DMA overlaps stage-0 projections.

Other changes vs v2:
 - exp runs on the UNMASKED diagonal scores (bounded: pass-1's row max is
   over legal keys, so masked entries exceed it by at most the score range
   ~O(10), well inside exp's fp32/bf16 range); the above-diagonal region of
   the bf16 P tile is then zeroed by gpsimd affine_select. This takes the
   DVE mask-add out of the S2->exp chain entirely.
 - all input DMAs are emitted up front, interleaved (weights mo-chunk,
   then the matching residual chunk) so stage 0 can start ~3us in.

Carried over from v2: fp16 activations/weights (bf16 P/V for exp range),
fused -max row in the S2 matmul (row 64 of qT/kT), stride-16 sampled-key
pass-1 row max (underestimates only; softmax shift-invariance makes any
shift exact), V^T computed with stationary weights then PE-transposed into
[k, d] slabs, AV riding 5 blocks behind S2, host-side reduce of the 4
per-batch partials with b_V/b_O folded into one bias row.
"""

import numpy as np

import concourse.bass as bass
import concourse.mybir as mybir
import concourse.tile as tile
from concourse import bacc
from concourse.bass_utils import run_bass_kernel_spmd
from concourse.masks import make_identity

dt = mybir.dt
AF = mybir.ActivationFunctionType
ALU = mybir.AluOpType
AX = mybir.AxisListType

NUM_HEADS = 16
D_MODEL = 1024
D_HEAD = 64
D_SEQ = 2048
BATCH = 2
N_CORES = 8
HPG = 4          # heads per group (per core)
G = 4            # groups per batch
SQ = 512         # seq chunk (pipeline stage)
MO = D_MODEL // 128   # 8 m-chunks
NQT = D_SEQ // 128    # 16 q tiles
NQC = D_SEQ // SQ     # 4 seq chunks / stages
STRIDE = 16           # pass-1 past-key subsample stride
NSAMP = (D_SEQ - 128) // STRIDE   # 120 sampled past keys max
AV_LAG = 5            # A*V rides this many blocks behind S2
# last head of each stage is even -> its attnT write is a direct DVE store
# (no DMA down), shortening the chain into the next stage's outproj
HEAD_ORDER = [1, 3, 0, 2]

_prog_cache = {}


def _build_program():
    nc = bacc.Bacc("TRN2", target_bir_lowering=False, debug=False,
                   num_devices=N_CORES)

    resT_in = nc.dram_tensor("resT", [128, NQC, MO, SQ], dt.float16, kind="ExternalInput").ap()
    wq_in = nc.dram_tensor("wq", [128, MO, 2, 128], dt.float16, kind="ExternalInput").ap()
    wk_in = nc.dram_tensor("wk", [128, MO, 2, 128], dt.float16, kind="ExternalInput").ap()
    wv_in = nc.dram_tensor("wv", [128, MO, 2, 128], dt.float16, kind="ExternalInput").ap()
    bq_in = nc.dram_tensor("bq", [128, 2], dt.float32, kind="ExternalInput").ap()
    bk_in = nc.dram_tensor("bk", [128, 2], dt.float32, kind="ExternalInput").ap()
    wo_in = nc.dram_tensor("wo", [128, 2, D_MODEL], dt.float16, kind="ExternalInput").ap()
    out_io = nc.dram_tensor("out_part", [D_SEQ, D_MODEL], dt.float32, kind="ExternalOutput").ap()

    with tile.TileContext(nc) as tc:
        from contextlib import ExitStack
        outer = ExitStack()
        with outer:
            const = outer.enter_context(tc.tile_pool(name="const", bufs=1))
            qkp = outer.enter_context(tc.tile_pool(name="qkp", bufs=1))
            vp = outer.enter_context(tc.tile_pool(name="vp", bufs=1))
            statp = outer.enter_context(tc.tile_pool(name="statp", bufs=1))
            mxsp = outer.enter_context(tc.tile_pool(name="mxsp", bufs=2))
            scrp = outer.enter_context(tc.tile_pool(name="scrp", bufs=3))
            rcp = outer.enter_context(tc.tile_pool(name="rcp", bufs=4))
            rcbp = outer.enter_context(tc.tile_pool(name="rcbp", bufs=4))
            ttp = outer.enter_context(tc.tile_pool(name="ttp", bufs=4))
            ptp = outer.enter_context(tc.tile_pool(name="ptp", bufs=22))
            atp = outer.enter_context(tc.tile_pool(name="atp", bufs=1))
            osp = outer.enter_context(tc.tile_pool(name="osp", bufs=3))
            rp = outer.enter_context(tc.tile_pool(name="rp", bufs=1))
            wp = outer.enter_context(tc.tile_pool(name="wp", bufs=1))
            qtp = outer.enter_context(tc.tile_pool(name="qtp", bufs=3))
            vtp = outer.enter_context(tc.tile_pool(name="vtp", bufs=4))
            psum = outer.enter_context(tc.tile_pool(name="psum", bufs=3, space="PSUM"))
            projps = outer.enter_context(tc.tile_pool(name="projps", bufs=2, space="PSUM"))
            p1ps = outer.enter_context(tc.tile_pool(name="p1ps", bufs=2, space="PSUM"))
            avps = outer.enter_context(tc.tile_pool(name="avps", bufs=1, space="PSUM"))
            dram = outer.enter_context(tc.tile_pool(name="dram", bufs=1, space="DRAM"))

            # ---- constants ----
            ident = const.tile([128, 128], dt.float32r, name="ident")
            ident_f = const.tile([128, 128], dt.float32, name="ident_f")
            make_identity(nc, ident_f[:])
            nc.vector.tensor_copy(ident[:], ident_f[:])
            ident_b = const.tile([128, 128], dt.bfloat16, name="ident_b")
            nc.vector.tensor_copy(ident_b[:], ident_f[:])

            # pass-1 diag mask (S [q, k] orientation: keep where j <= p)
            trimaskT = const.tile([128, 128 + NSAMP], dt.float32, name="trimaskT")
            nc.gpsimd.memset(trimaskT[:], 0.0)
            nc.gpsimd.affine_select(out=trimaskT[:, 0:128], in_=trimaskT[:, 0:128],
                                    compare_op=ALU.is_ge, fill=-1e30,
                                    base=0, pattern=[[-1, 128]], channel_multiplier=1)

            bqs = const.tile([128, 2], dt.float32, name="bqs")
            bks = const.tile([128, 2], dt.float32, name="bks")
            # issue the first gpsimd DMA and partition_broadcast immediately:
            # each triggers a one-time ~6us IRAM library load on the Q7 cores
            # that must overlap the startup DMA wait, not stall the gpsimd
            # queue mid-kernel
            nc.gpsimd.dma_start(bqs[:], bq_in[:])
            warmb = const.tile([64, 32], dt.float32, name="warmb")
            nc.gpsimd.partition_broadcast(warmb[:], ident_f[0:1, 0:32])
            nc.sync.dma_start(bks[:], bk_in[:])

            # dummy matmuls: keep the PE busy from ~1.5us so the HAM clock
            # gate reaches K=8/8 (2.4 GHz) before the first projection's
            # input DMA lands (~10us); otherwise the whole first projection
            # wave runs at 1.2 GHz
            wps = psum.tile([128, 512], dt.float32, name="warmps", tag="ps")
            for _ in range(36):
                nc.tensor.matmul(wps[:, 0:128], ident_b[:], ident_b[:],
                                 start=True, stop=True)
            warms = const.tile([1, 1], dt.float32, name="warms")
            nc.vector.tensor_copy(warms[:], wps[0:1, 0:1])
            # dummy exp: loads the ACT table set (~2.7us) during startup
            warme = const.tile([1, 1], dt.float32, name="warme")
            nc.scalar.activation(warme[:], ident_f[0:1, 0:1], AF.Exp)


            # ---- persistent activations ----
            qT = [qkp.tile([65, D_SEQ], dt.float16, name=f"qT{h}") for h in range(HPG)]
            kT = [qkp.tile([65, D_SEQ], dt.float16, name=f"kT{h}") for h in range(HPG)]
            kTs = [qkp.tile([64, NSAMP], dt.float16, name=f"kTs{h}") for h in range(HPG)]
            # V in [k, d] layout, one 65-wide slab per head: cols 0:64 = V_h, col 64 = 1.0
            vkd = vp.tile([128, NQT, HPG, 65], dt.bfloat16, name="vkd")

            attnT = atp.tile([128, 2, D_SEQ], dt.float16, name="attnT")
            wo = atp.tile([128, 2, D_MODEL], dt.float16, name="wo")

            # negmx[h][sc] col qt%4 = -(max over sampled+diag keys) per q row
            negmx = [[statp.tile([128, 32], dt.float32r, name=f"negmx{h}_{sc}")
                      for sc in range(NQC)] for h in range(HPG)]

            # ---- weights + residual: few big DMAs (the Sync engine spends
            # ~0.6us of issue time per DMA instruction, so DMA count matters
            # more than transfer size); residual is laid out per-stage
            # contiguous on the host so each stage is one descriptor run ----
            wq = wp.tile([128, MO, 2, 128], dt.float16, name="wq")
            wk = wp.tile([128, MO, 2, 128], dt.float16, name="wk")
            wv = wp.tile([128, MO, 2, 128], dt.float16, name="wv")
            resT = [rp.tile([128, MO, SQ], dt.float16, name=f"resT{sc}")
                    for sc in range(NQC)]
            # inputs split across BOTH hardware DGE queues (SP + Activation)
            # so the ~6.5MB load halves in time; the ACT engine only pays a
            # ~0.6us issue cost per DMA, long before its first real work
            nc.sync.dma_start(wq[:, 0:4], wq_in[:, 0:4])
            nc.scalar.dma_start(wk[:, 0:4], wk_in[:, 0:4])
            nc.sync.dma_start(resT[0][:, 0:4], resT_in[:, 0, 0:4])
            nc.scalar.dma_start(wk[:, 4:8], wk_in[:, 4:8])
            nc.sync.dma_start(wq[:, 4:8], wq_in[:, 4:8])
            nc.sync.dma_start(resT[0][:, 4:8], resT_in[:, 0, 4:8])
            nc.scalar.dma_start(wv[:], wv_in[:])
            nc.scalar.dma_start(wo[:], wo_in[:])
            nc.sync.dma_start(resT[1][:], resT_in[:, 1])
            nc.scalar.dma_start(resT[2][:], resT_in[:, 2])
            nc.sync.dma_start(resT[3][:], resT_in[:, 3])
            # persistent-tile memsets AFTER the warmup emission so the DVE
            # queue doesn't delay the ident casts the warmup matmuls need
            nc.vector.memset(vkd[:, :, :, 64], 1.0)
            for h in range(HPG):
                nc.vector.memset(kT[h][64:65, :], 1.0)

            # ================= stage building blocks =================

            def proj_half(sc, which, p, half):
                """Half a projection group: 4 accumulating matmuls; the
                second half also drains psum -> qT/kT/vT."""
                w, dst, bias = {
                    "q": (wq, qT, bqs), "k": (wk, kT, bks), "v": (wv, None, None),
                }[which]
                if half == 0:
                    ps = projps.tile([128, SQ], dt.float32,
                                     name=f"ps_{which}{p}_{sc}", tag="proj")
                    proj_half.live[(sc, which, p)] = ps
                else:
                    ps = proj_half.live.pop((sc, which, p))
                for mo in range(4 * half, 4 * half + 4):
                    nc.tensor.matmul(ps[:], w[:, mo, p, :], resT[sc][:, mo],
                                     start=(mo == 0), stop=(mo == MO - 1))
                if half == 0:
                    return
                cols = slice(sc * SQ, (sc + 1) * SQ)
                if which == "v":
                    vt = vtp.tile([128, SQ], dt.bfloat16, name=f"vT{p}_{sc}", tag="vt")
                    nc.scalar.copy(vt[:], ps[:])
                    proj_half.vt[(sc, p)] = vt
                    return
                # even head: direct ACT with bias
                nc.scalar.activation(dst[2 * p][0:64, cols], ps[0:64, :], AF.Identity,
                                     bias=bias[0:64, p:p + 1], scale=1.0)
                # odd head: aligned ACT into tmp rows 64:128, then DMA down
                qt_t = qtp.tile([128, SQ], dt.float16, name=f"qtmp_{which}{p}{sc}", tag="qtmp")
                nc.scalar.activation(qt_t[64:128, :], ps[64:128, :], AF.Identity,
                                     bias=bias[64:128, p:p + 1], scale=1.0)
                nc.gpsimd.dma_start(dst[2 * p + 1][0:64, cols], qt_t[64:128, :])
            proj_half.live = {}
            proj_half.vt = {}

            def kts_copy(sc, h):
                # sampled keys (stride 16) newly available from seq chunk sc
                lo = sc * SQ // STRIDE
                hi = min((sc + 1) * SQ, D_SEQ - 128) // STRIDE
                if hi <= lo:
                    return
                nc.vector.tensor_copy(
                    kTs[h][:, lo:hi],
                    kT[h][0:64, sc * SQ:hi * STRIDE:STRIDE])

            def vt_transpose(sc, kc_local):
                kc = 4 * sc + kc_local
                ps = psum.tile([128, 512], dt.float32, name=f"ps_v{kc}", tag="ps")
                pb = ps[:].bitcast(dt.bfloat16)
                for c in range(2):
                    nc.tensor.transpose(pb[:, 512 * c:512 * c + 128],
                                        proj_half.vt[(sc, c)][:, kc_local * 128:(kc_local + 1) * 128],
                                        ident_b[:])
                nc.vector.tensor_copy(
                    vkd[:, kc, :, 0:64].rearrange("p (c h) d -> p c h d", c=2),
                    pb.rearrange("p (c r) -> p c r", c=2)[:, :, 0:128]
                      .rearrange("p c (h d) -> p c h d", h=2))

            def p1_block(h, qt):
                nsamp = (qt * 128) // STRIDE
                ncols = 128 + nsamp
                ps = p1ps.tile([128, 512], dt.float32, name=f"ps_p1_{h}_{qt}", tag="p1")
                qstat = qT[h][0:64, qt * 128:(qt + 1) * 128]
                nc.tensor.matmul(ps[:, 0:128], qstat,
                                 kT[h][0:64, qt * 128:(qt + 1) * 128],
                                 start=True, stop=True)
                if nsamp:
                    nc.tensor.matmul(ps[:, 128:ncols], qstat,
                                     kTs[h][:, 0:nsamp], start=True, stop=True)
                scr = scrp.tile([128, 128 + NSAMP], dt.bfloat16,
                                name=f"scr{h}_{qt}", tag="scr")
                nc.vector.tensor_tensor(scr[:, 0:ncols], ps[:, 0:ncols],
                                        trimaskT[:, 0:ncols], ALU.add)
                nc.vector.tensor_reduce(negmx[h][qt // 4][:, (qt % 4):(qt % 4) + 1],
                                        scr[:, 0:ncols], AX.X, ALU.max, negate=True)

            def p1_final(h, sc):
                # -max column -> row 64 of qT[h] for this chunk (via transpose
                # on the PE and a DRAM partition->row reshape hop)
                pst = psum.tile([128, 512], dt.float32r, name=f"ps_tp{h}_{sc}", tag="ps")
                nc.tensor.transpose(pst[0:32, 0:128], negmx[h][sc][:], ident[:])
                mxs = mxsp.tile([4, 128], dt.float16, name=f"mxs{h}_{sc}", tag="mxs")
                nc.vector.tensor_copy(mxs[:], pst[0:4, 0:128].bitcast(dt.float32))
                mrow = dram.tile([4, 128], dt.float16, name=f"mrow{h}_{sc}")
                nc.sync.dma_start(mrow[:], mxs[:])
                nc.sync.dma_start(qT[h][64:65, sc * SQ:(sc + 1) * SQ],
                                  mrow[:].rearrange("t f -> (t f)").unsqueeze(0))

            def outproj_sub(qc, sub):
                st = 4 * qc + sub
                final = (qc == NQC - 1)
                osb = osp.tile([128, D_MODEL], dt.float32, name=f"osb{st}", tag="osb")
                for mc in range(2):
                    ps = psum.tile([128, 512], dt.float32, name=f"ps_o{st}_{mc}", tag="ps")
                    for eo in range(2):
                        nc.tensor.matmul(ps[:], attnT[:, eo, st * 128:(st + 1) * 128],
                                         wo[:, eo, mc * 512:(mc + 1) * 512],
                                         start=(eo == 0), stop=(eo == 1))
                    # the final chunk drains via ACT (idle after the last exp)
                    # so its psum copies don't serialize with the matmuls
                    if final:
                        nc.scalar.copy(osb[:, mc * 512:(mc + 1) * 512], ps[:])
                    else:
                        nc.vector.tensor_copy(osb[:, mc * 512:(mc + 1) * 512], ps[:])
                # final-chunk output alternates the two HWDGE queues so the
                # last transfers run in parallel instead of serializing
                eng = (nc.sync if sub % 2 == 0 else nc.scalar) if final else nc.gpsimd
                eng.dma_start(out_io[st * 128:(st + 1) * 128, :], osb[:])

            # ---- filler machinery: work for stage sc+1, emitted inside
            # the attention stream of stage sc ----
            def make_filler(sc):
                """Filler items that prepare stage sc (emitted during stage
                sc-1's attention). Pass-1 comes right after the q/k
                projections so its DMA-hop latency hides behind the V
                projections / transposes; outproj has no dependencies on this
                stage and goes last as pure PE filler."""
                items = []
                for which, p in (("q", 0), ("q", 1), ("k", 0), ("k", 1)):
                    for half in range(2):
                        items.append(lambda w=which, p=p, hf=half: proj_half(sc, w, p, hf))
                for h in HEAD_ORDER:
                    items.append(lambda h=h: kts_copy(sc, h))
                # V before pass-1: its DVE psum drain must not queue behind
                # pass-1's DVE adds/reduces (the V transposes need it)
                for p in range(2):
                    for half in range(2):
                        items.append(lambda p=p, hf=half: proj_half(sc, "v", p, hf))
                for kcl in range(4):
                    items.append(lambda kcl=kcl: vt_transpose(sc, kcl))
                for h in HEAD_ORDER:
                    for qt in range(4 * sc, 4 * sc + 4):
                        items.append(lambda h=h, qt=qt: p1_block(h, qt))
                    items.append(lambda h=h: p1_final(h, sc))
                # outproj last: pure PE work hiding the pass-1 DMA-hop latency
                if sc >= 2:
                    for sub in range(4):
                        items.append(lambda sub=sub: outproj_sub(sc - 2, sub))
                return items

            class Filler:
                def __init__(self, items, slots):
                    self.items = items
                    self.per = len(items) / max(1, slots)
                    self.budget = 0.0
                    self.idx = 0

                def step(self, mult=1.0):
                    self.budget += self.per * mult
                    while self.idx < min(len(self.items), int(self.budget)):
                        self.items[self.idx]()
                        self.idx += 1

                def drain(self):
                    while self.idx < len(self.items):
                        self.items[self.idx]()
                        self.idx += 1

            # ================= stage 0 preamble =================
            for which, p in (("q", 0), ("q", 1), ("k", 0), ("k", 1)):
                for half in range(2):
                    proj_half(0, which, p, half)
            for h in HEAD_ORDER:
                kts_copy(0, h)
            for p in range(2):
                for half in range(2):
                    proj_half(0, "v", p, half)
            for kcl in range(4):
                vt_transpose(0, kcl)
            for h in HEAD_ORDER:
                for qt in range(4):
                    p1_block(h, qt)
                p1_final(h, 0)

            # ================= pipelined attention stages =================
            for sc in range(NQC):
                nkt = 4 * sc + 4
                # stage 3 keeps outproj(2) for the exp-throttled gaps of its
                # last head (emitted there directly) instead of generic filler
                fill = Filler(make_filler(sc + 1) if sc + 1 < NQC else [],
                              slots=HPG * (nkt + 1))
                for h in HEAD_ORDER:
                    # emit filler ahead of the first S2 so the PE queue has
                    # work while the S2 waits on cross-engine dependencies
                    fill.step()
                    pt_blks = {}
                    av = avps.tile([128, 512], dt.float32, name=f"ps_av_{h}_{sc}", tag="av")

                    def emit_av(kt, av=av, pt_blks=pt_blks, sc=sc, h=h, nkt=nkt):
                        # diag blocks: cols < 128r are fully masked (zero in
                        # pt), so skip them — earlier kt blocks own those q's
                        r = max(0, kt - 4 * sc)
                        nc.tensor.matmul(av[0:65, 128 * r:], vkd[:, kt, h, :],
                                         pt_blks[kt][:, 128 * r:],
                                         start=(kt == 0), stop=(kt == nkt - 1),
                                         skip_group_check=True)

                    for kt in range(nkt):
                        ps = psum.tile([128, 512], dt.float32, name=f"ps_s2_{h}_{sc}_{kt}", tag="ps")
                        r = max(0, kt - 4 * sc)
                        nc.tensor.matmul(ps[:, 128 * r:],
                                         kT[h][:, kt * 128:(kt + 1) * 128],
                                         qT[h][:, sc * SQ + 128 * r:(sc + 1) * SQ],
                                         start=True, stop=True)
                        pt = ptp.tile([128, 512], dt.bfloat16, name=f"pt{h}_{sc}_{kt}", tag="pt")
                        nc.scalar.activation(pt[:, 128 * r:], ps[:, 128 * r:], AF.Exp)
                        if kt - 4 * sc >= 0:
                            # zero above-diagonal of the exp'd diag sub-block
                            nc.gpsimd.affine_select(
                                out=pt[:, 128 * r:128 * r + 128],
                                in_=pt[:, 128 * r:128 * r + 128],
                                compare_op=ALU.is_ge, fill=0.0,
                                base=0, pattern=[[1, 128]], channel_multiplier=-1)
                        pt_blks[kt] = pt
                        if kt >= AV_LAG:
                            emit_av(kt - AV_LAG)
                        fill.step()
                        if sc == NQC - 1 and h == HEAD_ORDER[-1] and kt % 4 == 3:
                            outproj_sub(NQC - 2, kt // 4)
                    for kt in range(max(0, nkt - AV_LAG), nkt):
                        emit_av(kt)
                    # normalize: broadcast the Z row to 64 partitions, then a
                    # full-rate approximate reciprocal (51 ULP, plenty for the
                    # softmax denominator) — no DMA round trips in the chain
                    zsb = rcp.tile([65, 512], dt.float32, name=f"zsb{h}_{sc}", tag="zsb", bufs=4)
                    nc.vector.tensor_copy(zsb[:], av[0:65, :])
                    # one row DMA to move Z to partition 0 (partition_broadcast
                    # broadcasts partition 0; engines cannot shift partitions)
                    zrow = rcp.tile([1, 512], dt.float32, name=f"zrow{h}_{sc}", tag="zrow", bufs=4)
                    nc.sync.dma_start(zrow[:], zsb[64:65, :])
                    zbc = rcbp.tile([64, 512], dt.float32, name=f"zbc{h}_{sc}", tag="zbc")
                    nc.gpsimd.partition_broadcast(zbc[:], zrow[:])
                    rcb = rcbp.tile([64, 512], dt.float32, name=f"rcb{h}_{sc}", tag="rcb")
                    nc.vector.reciprocal_approx_fast(rcb[:], zbc[:])
                    eh = h // 2
                    last = (sc == NQC - 1 and h == HEAD_ORDER[-1])
                    if last:
                        # dummy matmuls bridge the PE over the Z-chain latency
                        # so the HAM clock gate stays at 2.4 GHz for the final
                        # output projection
                        wp2 = psum.tile([128, 512], dt.float32, name="warmps2", tag="ps")
                        for _ in range(24):
                            nc.tensor.matmul(wp2[:, 0:128], ident_b[:], ident_b[:],
                                             start=True, stop=True)
                    if h % 2 == 0:
                        if last:
                            # final head of the kernel: normalize in 128-col
                            # pieces, launching each outproj subtile as soon
                            # as its columns land, to shorten the drain tail
                            for st in range(4):
                                cols = slice(sc * SQ + st * 128, sc * SQ + (st + 1) * 128)
                                nc.vector.tensor_tensor(attnT[0:64, eh, cols],
                                                        zsb[0:64, st * 128:(st + 1) * 128],
                                                        rcb[:, st * 128:(st + 1) * 128],
                                                        ALU.mult)
                                outproj_sub(sc, st)
                        else:
                            nc.vector.tensor_tensor(attnT[0:64, eh, sc * SQ:(sc + 1) * SQ],
                                                    zsb[0:64, :], rcb[:], ALU.mult)
                    else:
                        att = ttp.tile([64, 512], dt.float16, name=f"att{h}_{sc}", tag="att")
                        nc.vector.tensor_tensor(att[:], zsb[0:64, :], rcb[:], ALU.mult)
                        nc.gpsimd.dma_start(attnT[64:128, eh, sc * SQ:(sc + 1) * SQ], att[:])
                fill.drain()
            nc.vector.tensor_copy(warms[:], wp2[0:1, 0:1])

    nc.compile()
    return nc


def _get_program():
    if "nc" not in _prog_cache:
        _prog_cache["nc"] = _build_program()
    return _prog_cache["nc"]


def _shard_inputs(residual, W_Q, W_K, W_V, W_O, b_Q, b_K, b_V, b_O):
    f32 = np.float32
    in_maps = []
    for core in range(N_CORES):
        b, g = core // G, core % G
        heads = list(range(HPG * g, HPG * g + HPG))
        # residual^T: [m, s] -> [mi, sc, mo, s-chunk] (per-stage contiguous)
        rT = np.ascontiguousarray(
            residual[b].T.reshape(MO, 128, NQC, SQ).transpose(1, 2, 0, 3)).astype(np.float16)

        def wstack(W, scale=1.0):
            # per pair p: [m, 128] -> [mi, mo, p, 128]
            pairs = []
            for p in range(2):
                wpair = np.concatenate([W[heads[2 * p]], W[heads[2 * p + 1]]], axis=1) * scale
                pairs.append(wpair.reshape(MO, 128, 128).transpose(1, 0, 2))
            return np.ascontiguousarray(np.stack(pairs, axis=2)).astype(np.float16)

        wq = wstack(W_Q, 0.125)
        wk = wstack(W_K)
        wv = wstack(W_V)
        bq = np.stack([np.concatenate([b_Q[heads[2 * p]], b_Q[heads[2 * p + 1]]]) * 0.125
                       for p in range(2)], axis=1).astype(f32)
        bk = np.stack([np.concatenate([b_K[heads[2 * p]], b_K[heads[2 * p + 1]]])
                       for p in range(2)], axis=1).astype(f32)
        wo = np.ascontiguousarray(
            W_O[256 * g:256 * (g + 1)].reshape(2, 128, D_MODEL).transpose(1, 0, 2)
        ).astype(np.float16)
        in_maps.append(dict(resT=rT, wq=wq, wk=wk, wv=wv, bq=np.ascontiguousarray(bq),
                            bk=np.ascontiguousarray(bk), wo=wo))
    return in_maps


def _run(inputs, trace=False):
    nc = _get_program()
    in_maps = _shard_inputs(**inputs)
    res = run_bass_kernel_spmd(nc, in_maps, core_ids=list(range(N_CORES)), trace=trace)
    # host-side reduce over the 4 head-group cores of each batch + folded bias
    W_O, b_V, b_O = inputs["W_O"], inputs["b_V"], inputs["b_O"]
    c = (b_O + sum(b_V[h] @ W_O[64 * h:64 * (h + 1)] for h in range(NUM_HEADS))
         ).astype(np.float32)
    out = np.empty((BATCH, D_SEQ, D_MODEL), np.float32)
    for b in range(BATCH):
        acc = res.results[b * G]["out_part"].astype(np.float32).copy()
        for g in range(1, G):
            acc += res.results[b * G + g]["out_part"]
        out[b] = acc + c[None, :]
    return out, res


def kernel(**inputs):
    out, _ = _run(inputs, trace=False)
    return out


# revision 55
# speedup vs baseline: 1.0438x; 1.0025x over previous
"""Trainium2 Bass kernel for causal multi-head attention (nn_Attention_3161095930536).

Model: batch=2, seq=2048, d_model=1024, 16 heads x 64. Reference computes
QKV projections + causal softmax attention + output projection (+ biases).

Sharding over 8 NeuronCores: core = (batch b = core//4) x (head-group
g = core%4, 4 heads each). Each core computes its head-group's attention and
a partial output projection into DRAM; the HOST sums the 4 partials per batch
(and adds the folded output bias), keeping the device critical path free of
collectives.

v3: single software-pipelined stream over the 4 seq chunks (sc). The v2
two-phase design (project everything, then attend) left the PE idle in
bursts during the early attention chunks, dropping the HAM clock gate to
K=4/8 (1.2 GHz) for ~90us of the kernel, and serialized ~30us of input DMA
at the start. Here the projections / pass-1 / V transposes for chunk sc+1
are emitted as *filler* inside the attention stream of chunk sc, so:
 - the PE has dense matmul work end to end (HAM stays at 2.4 GHz),
 - the projection-phase ACT work (bias adds) overlaps the exp stream,
 - the startup residual DMA overlaps stage-0 projections.

Other changes vs v2 (each validated against a perfetto trace):
 - exp runs on the UNMASKED diagonal scores (bounded: pass-1's row max is
   over legal keys, so masked entries exceed it by at most the score range
   ~O(10), well inside exp's fp32/bf16 range); the above-diagonal region of
   the bf16 P tile is then zeroed by gpsimd affine_select. This takes the
   DVE mask-add out of the S2->exp chain entirely.
 - input DMAs are emitted up front as FEW, LARGE transfers (the Sync engine
   pays ~0.6us of issue time per DMA instruction) split across both HWDGE
   queues (SP + Activation), with the residual laid out per-stage-contiguous
   on the host; latency-tolerant hops (odd-head qT/attnT stores, partial-out
   stores) go through the gpsimd SWDGE queue instead of Sync.
 - softmax normalize: one row-DMA moves Z to partition 0, then
   gpsimd partition_broadcast + DVE reciprocal_approx_fast (51-ULP, full
   rate) -- the v2 row->column->row DMA round trip (and its 2x ~2.6us
   SBUF-DMA completion latency) is gone. DVE's exact reciprocal is an
   8-cycle/element iterative op and must never touch a [1,512] row.
 - warmup: dummy ident matmuls + a dummy exp + a dummy gpsimd DMA/broadcast
   at t~0 pull the HAM clock-gate warmup, the ACT exp-table load (~2.7us)
   and the two Q7 IRAM library loads (~6us each) into the startup DMA
   shadow; another dummy-matmul bridge keeps the PE warm across the final
   head's normalize chain so the last output projection runs at 2.4 GHz.
 - the final chunk's output projection is emitted piecewise behind the last
   head's normalize (128 cols at a time), drains psum via ACT (idle after
   the last exp), and alternates its output DMAs across both HWDGE queues.

Carried over from v2: fp16 activations/weights (bf16 P/V for exp range),
fused -max row in the S2 matmul (row 64 of qT/kT), stride-16 sampled-key
pass-1 row max (underestimates only; softmax shift-invariance makes any
shift exact), V^T computed with stationary weights then PE-transposed into
[k, d] slabs, AV riding 5 blocks behind S2, host-side reduce of the 4
per-batch partials with b_V/b_O folded into one bias row.
"""

import numpy as np

import concourse.bass as bass
import concourse.mybir as mybir
import concourse.tile as tile
from concourse import bacc
from concourse.bass_utils import run_bass_kernel_spmd
from concourse.masks import make_identity

dt = mybir.dt
AF = mybir.ActivationFunctionType
ALU = mybir.AluOpType
AX = mybir.AxisListType

NUM_HEADS = 16
D_MODEL = 1024
D_HEAD = 64
D_SEQ = 2048
BATCH = 2
N_CORES = 8
HPG = 4          # heads per group (per core)
G = 4            # groups per batch
SQ = 512         # seq chunk (pipeline stage)
MO = D_MODEL // 128   # 8 m-chunks
NQT = D_SEQ // 128    # 16 q tiles
NQC = D_SEQ // SQ     # 4 seq chunks / stages
STRIDE = 16           # pass-1 past-key subsample stride
NSAMP = (D_SEQ - 128) // STRIDE   # 120 sampled past keys max
AV_LAG = 5            # A*V rides this many blocks behind S2
# last head of each stage is even -> its attnT write is a direct DVE store
# (no DMA down), shortening the chain into the next stage's outproj
HEAD_ORDER = [1, 3, 0, 2]

_prog_cache = {}


def _build_program():
    nc = bacc.Bacc("TRN2", target_bir_lowering=False, debug=False,
                   num_devices=N_CORES)

    resT_in = nc.dram_tensor("resT", [128, NQC, MO, SQ], dt.float16, kind="ExternalInput").ap()
    wq_in = nc.dram_tensor("wq", [128, MO, 2, 128], dt.float16, kind="ExternalInput").ap()
    wk_in = nc.dram_tensor("wk", [128, MO, 2, 128], dt.float16, kind="ExternalInput").ap()
    wv_in = nc.dram_tensor("wv", [128, MO, 2, 128], dt.float16, kind="ExternalInput").ap()
    bq_in = nc.dram_tensor("bq", [128, 2], dt.float32, kind="ExternalInput").ap()
    bk_in = nc.dram_tensor("bk", [128, 2], dt.float32, kind="ExternalInput").ap()
    wo_in = nc.dram_tensor("wo", [128, 2, D_MODEL], dt.float16, kind="ExternalInput").ap()
    out_io = nc.dram_tensor("out_part", [D_SEQ, D_MODEL], dt.float32, kind="ExternalOutput").ap()

    with tile.TileContext(nc) as tc:
        from contextlib import ExitStack
        outer = ExitStack()
        with outer:
            const = outer.enter_context(tc.tile_pool(name="const", bufs=1))
            qkp = outer.enter_context(tc.tile_pool(name="qkp", bufs=1))
            vp = outer.enter_context(tc.tile_pool(name="vp", bufs=1))
            statp = outer.enter_context(tc.tile_pool(name="statp", bufs=1))
            mxsp = outer.enter_context(tc.tile_pool(name="mxsp", bufs=2))
            scrp = outer.enter_context(tc.tile_pool(name="scrp", bufs=3))
            rcp = outer.enter_context(tc.tile_pool(name="rcp", bufs=4))
            rcbp = outer.enter_context(tc.tile_pool(name="rcbp", bufs=4))
            ttp = outer.enter_context(tc.tile_pool(name="ttp", bufs=4))
            ptp = outer.enter_context(tc.tile_pool(name="ptp", bufs=22))
            atp = outer.enter_context(tc.tile_pool(name="atp", bufs=1))
            osp = outer.enter_context(tc.tile_pool(name="osp", bufs=3))
            rp = outer.enter_context(tc.tile_pool(name="rp", bufs=1))
            wp = outer.enter_context(tc.tile_pool(name="wp", bufs=1))
            qtp = outer.enter_context(tc.tile_pool(name="qtp", bufs=3))
            vtp = outer.enter_context(tc.tile_pool(name="vtp", bufs=4))
            psum = outer.enter_context(tc.tile_pool(name="psum", bufs=3, space="PSUM"))
            projps = outer.enter_context(tc.tile_pool(name="projps", bufs=2, space="PSUM"))
            p1ps = outer.enter_context(tc.tile_pool(name="p1ps", bufs=2, space="PSUM"))
            avps = outer.enter_context(tc.tile_pool(name="avps", bufs=1, space="PSUM"))
            dram = outer.enter_context(tc.tile_pool(name="dram", bufs=1, space="DRAM"))

            # ---- constants ----
            ident = const.tile([128, 128], dt.float32r, name="ident")
            ident_f = const.tile([128, 128], dt.float32, name="ident_f")
            make_identity(nc, ident_f[:])
            nc.vector.tensor_copy(ident[:], ident_f[:])
            ident_b = const.tile([128, 128], dt.bfloat16, name="ident_b")
            nc.vector.tensor_copy(ident_b[:], ident_f[:])

            # pass-1 diag mask (S [q, k] orientation: keep where j <= p)
            trimaskT = const.tile([128, 128 + NSAMP], dt.float32, name="trimaskT")
            nc.gpsimd.memset(trimaskT[:], 0.0)
            nc.gpsimd.affine_select(out=trimaskT[:, 0:128], in_=trimaskT[:, 0:128],
                                    compare_op=ALU.is_ge, fill=-1e30,
                                    base=0, pattern=[[-1, 128]], channel_multiplier=1)

            bqs = const.tile([128, 2], dt.float32, name="bqs")
            bks = const.tile([128, 2], dt.float32, name="bks")
            # issue the first gpsimd DMA and partition_broadcast immediately:
            # each triggers a one-time ~6us IRAM library load on the Q7 cores
            # that must overlap the startup DMA wait, not stall the gpsimd
            # queue mid-kernel
            nc.gpsimd.dma_start(bqs[:], bq_in[:])
            warmb = const.tile([64, 32], dt.float32, name="warmb")
            nc.gpsimd.partition_broadcast(warmb[:], ident_f[0:1, 0:32])
            nc.sync.dma_start(bks[:], bk_in[:])

            # dummy matmuls: keep the PE busy from ~1.5us so the HAM clock
            # gate reaches K=8/8 (2.4 GHz) before the first projection's
            # input DMA lands (~10us); otherwise the whole first projection
            # wave runs at 1.2 GHz
            wps = psum.tile([128, 512], dt.float32, name="warmps", tag="ps")
            for _ in range(36):
                nc.tensor.matmul(wps[:, 0:128], ident_b[:], ident_b[:],
                                 start=True, stop=True)
            warms = const.tile([1, 1], dt.float32, name="warms")
            nc.vector.tensor_copy(warms[:], wps[0:1, 0:1])
            # dummy exp: loads the ACT table set (~2.7us) during startup
            warme = const.tile([1, 1], dt.float32, name="warme")
            nc.scalar.activation(warme[:], ident_f[0:1, 0:1], AF.Exp)


            # ---- persistent activations ----
            qT = [qkp.tile([65, D_SEQ], dt.float16, name=f"qT{h}") for h in range(HPG)]
            kT = [qkp.tile([65, D_SEQ], dt.float16, name=f"kT{h}") for h in range(HPG)]
            kTs = [qkp.tile([64, NSAMP], dt.float16, name=f"kTs{h}") for h in range(HPG)]
            # V in [k, d] layout, one 65-wide slab per head: cols 0:64 = V_h, col 64 = 1.0
            vkd = vp.tile([128, NQT, HPG, 65], dt.bfloat16, name="vkd")

            attnT = atp.tile([128, 2, D_SEQ], dt.float16, name="attnT")
            wo = atp.tile([128, 2, D_MODEL], dt.float16, name="wo")

            # negmx[h][sc] col qt%4 = -(max over sampled+diag keys) per q row
            negmx = [[statp.tile([128, 32], dt.float32r, name=f"negmx{h}_{sc}")
                      for sc in range(NQC)] for h in range(HPG)]

            # ---- weights + residual: few big DMAs (the Sync engine spends
            # ~0.6us of issue time per DMA instruction, so DMA count matters
            # more than transfer size); residual is laid out per-stage
            # contiguous on the host so each stage is one descriptor run ----
            wq = wp.tile([128, MO, 2, 128], dt.float16, name="wq")
            wk = wp.tile([128, MO, 2, 128], dt.float16, name="wk")
            wv = wp.tile([128, MO, 2, 128], dt.float16, name="wv")
            resT = [rp.tile([128, MO, SQ], dt.float16, name=f"resT{sc}")
                    for sc in range(NQC)]
            # inputs split across BOTH hardware DGE queues (SP + Activation)
            # so the ~6.5MB load halves in time; the ACT engine only pays a
            # ~0.6us issue cost per DMA, long before its first real work
            nc.sync.dma_start(wq[:, 0:4], wq_in[:, 0:4])
            nc.scalar.dma_start(wk[:, 0:4], wk_in[:, 0:4])
            nc.sync.dma_start(resT[0][:, 0:4], resT_in[:, 0, 0:4])
            nc.scalar.dma_start(wk[:, 4:8], wk_in[:, 4:8])
            nc.sync.dma_start(wq[:, 4:8], wq_in[:, 4:8])
            nc.sync.dma_start(resT[0][:, 4:8], resT_in[:, 0, 4:8])
            nc.scalar.dma_start(wv[:], wv_in[:])
            nc.scalar.dma_start(wo[:], wo_in[:])
            nc.sync.dma_start(resT[1][:], resT_in[:, 1])
            nc.scalar.dma_start(resT[2][:], resT_in[:, 2])
            nc.sync.dma_start(resT[3][:], resT_in[:, 3])
            # persistent-tile memsets AFTER the warmup emission so the DVE
            # queue doesn't delay the ident casts the warmup matmuls need
            nc.vector.memset(vkd[:, :, :, 64], 1.0)
            for h in range(HPG):
                nc.vector.memset(kT[h][64:65, :], 1.0)

            # ================= stage building blocks =================

            def proj_half(sc, which, p, half):
                """Half a projection group: 4 accumulating matmuls; the
                second half also drains psum -> qT/kT/vT."""
                w, dst, bias = {
                    "q": (wq, qT, bqs), "k": (wk, kT, bks), "v": (wv, None, None),
                }[which]
                if half == 0:
                    ps = projps.tile([128, SQ], dt.float32,
                                     name=f"ps_{which}{p}_{sc}", tag="proj")
                    proj_half.live[(sc, which, p)] = ps
                else:
                    ps = proj_half.live.pop((sc, which, p))
                for mo in range(4 * half, 4 * half + 4):
                    nc.tensor.matmul(ps[:], w[:, mo, p, :], resT[sc][:, mo],
                                     start=(mo == 0), stop=(mo == MO - 1))
                if half == 0:
                    return
                cols = slice(sc * SQ, (sc + 1) * SQ)
                if which == "v":
                    vt = vtp.tile([128, SQ], dt.bfloat16, name=f"vT{p}_{sc}", tag="vt")
                    nc.scalar.copy(vt[:], ps[:])
                    proj_half.vt[(sc, p)] = vt
                    return
                # even head: direct ACT with bias
                nc.scalar.activation(dst[2 * p][0:64, cols], ps[0:64, :], AF.Identity,
                                     bias=bias[0:64, p:p + 1], scale=1.0)
                # odd head: aligned ACT into tmp rows 64:128, then DMA down
                qt_t = qtp.tile([128, SQ], dt.float16, name=f"qtmp_{which}{p}{sc}", tag="qtmp")
                nc.scalar.activation(qt_t[64:128, :], ps[64:128, :], AF.Identity,
                                     bias=bias[64:128, p:p + 1], scale=1.0)
                nc.gpsimd.dma_start(dst[2 * p + 1][0:64, cols], qt_t[64:128, :])
            proj_half.live = {}
            proj_half.vt = {}

            def kts_copy(sc, h):
                # sampled keys (stride 16) newly available from seq chunk sc
                lo = sc * SQ // STRIDE
                hi = min((sc + 1) * SQ, D_SEQ - 128) // STRIDE
                if hi <= lo:
                    return
                nc.vector.tensor_copy(
                    kTs[h][:, lo:hi],
                    kT[h][0:64, sc * SQ:hi * STRIDE:STRIDE])

            def vt_transpose(sc, kc_local):
                kc = 4 * sc + kc_local
                ps = psum.tile([128, 512], dt.float32, name=f"ps_v{kc}", tag="ps")
                pb = ps[:].bitcast(dt.bfloat16)
                for c in range(2):
                    nc.tensor.transpose(pb[:, 512 * c:512 * c + 128],
                                        proj_half.vt[(sc, c)][:, kc_local * 128:(kc_local + 1) * 128],
                                        ident_b[:])
                nc.vector.tensor_copy(
                    vkd[:, kc, :, 0:64].rearrange("p (c h) d -> p c h d", c=2),
                    pb.rearrange("p (c r) -> p c r", c=2)[:, :, 0:128]
                      .rearrange("p c (h d) -> p c h d", h=2))

            def p1_block(h, qt):
                nsamp = (qt * 128) // STRIDE
                ncols = 128 + nsamp
                ps = p1ps.tile([128, 512], dt.float32, name=f"ps_p1_{h}_{qt}", tag="p1")
                qstat = qT[h][0:64, qt * 128:(qt + 1) * 128]
                nc.tensor.matmul(ps[:, 0:128], qstat,
                                 kT[h][0:64, qt * 128:(qt + 1) * 128],
                                 start=True, stop=True)
                if nsamp:
                    nc.tensor.matmul(ps[:, 128:ncols], qstat,
                                     kTs[h][:, 0:nsamp], start=True, stop=True)
                scr = scrp.tile([128, 128 + NSAMP], dt.bfloat16,
                                name=f"scr{h}_{qt}", tag="scr")
                nc.vector.tensor_tensor(scr[:, 0:ncols], ps[:, 0:ncols],
                                        trimaskT[:, 0:ncols], ALU.add)
                nc.vector.tensor_reduce(negmx[h][qt // 4][:, (qt % 4):(qt % 4) + 1],
                                        scr[:, 0:ncols], AX.X, ALU.max, negate=True)

            def p1_final(h, sc):
                # -max column -> row 64 of qT[h] for this chunk (via transpose
                # on the PE and a DRAM partition->row reshape hop)
                pst = psum.tile([128, 512], dt.float32r, name=f"ps_tp{h}_{sc}", tag="ps")
                nc.tensor.transpose(pst[0:32, 0:128], negmx[h][sc][:], ident[:])
                mxs = mxsp.tile([4, 128], dt.float16, name=f"mxs{h}_{sc}", tag="mxs")
                nc.vector.tensor_copy(mxs[:], pst[0:4, 0:128].bitcast(dt.float32))
                mrow = dram.tile([4, 128], dt.float16, name=f"mrow{h}_{sc}")
                nc.sync.dma_start(mrow[:], mxs[:])
                nc.sync.dma_start(qT[h][64:65, sc * SQ:(sc + 1) * SQ],
                                  mrow[:].rearrange("t f -> (t f)").unsqueeze(0))

            def outproj_sub(qc, sub):
                st = 4 * qc + sub
                final = (qc == NQC - 1)
                osb = osp.tile([128, D_MODEL], dt.float32, name=f"osb{st}", tag="osb")
                for mc in range(2):
                    ps = psum.tile([128, 512], dt.float32, name=f"ps_o{st}_{mc}", tag="ps")
                    for eo in range(2):
                        nc.tensor.matmul(ps[:], attnT[:, eo, st * 128:(st + 1) * 128],
                                         wo[:, eo, mc * 512:(mc + 1) * 512],
                                         start=(eo == 0), stop=(eo == 1))
                    # the final chunk drains via ACT (idle after the last exp)
                    # so its psum copies don't serialize with the matmuls
                    if final:
                        nc.scalar.copy(osb[:, mc * 512:(mc + 1) * 512], ps[:])
                    else:
                        nc.vector.tensor_copy(osb[:, mc * 512:(mc + 1) * 512], ps[:])
                # final-chunk output alternates the two HWDGE queues so the
                # last transfers run in parallel instead of serializing
                eng = (nc.sync if sub % 2 == 0 else nc.scalar) if final else nc.gpsimd
                eng.dma_start(out_io[st * 128:(st + 1) * 128, :], osb[:])

            # ---- filler machinery: work for stage sc+1, emitted inside
            # the attention stream of stage sc ----
            def make_filler(sc):
                """Filler items that prepare stage sc (emitted during stage
                sc-1's attention). Pass-1 comes right after the q/k
                projections so its DMA-hop latency hides behind the V
                projections / transposes; outproj has no dependencies on this
                stage and goes last as pure PE filler."""
                items = []
                for which, p in (("q", 0), ("q", 1), ("k", 0), ("k", 1)):
                    for half in range(2):
                        items.append(lambda w=which, p=p, hf=half: proj_half(sc, w, p, hf))
                for h in HEAD_ORDER:
                    items.append(lambda h=h: kts_copy(sc, h))
                # V before pass-1: its DVE psum drain must not queue behind
                # pass-1's DVE adds/reduces (the V transposes need it)
                for p in range(2):
                    for half in range(2):
                        items.append(lambda p=p, hf=half: proj_half(sc, "v", p, hf))
                for kcl in range(4):
                    items.append(lambda kcl=kcl: vt_transpose(sc, kcl))
                for h in HEAD_ORDER:
                    for qt in range(4 * sc, 4 * sc + 4):
                        items.append(lambda h=h, qt=qt: p1_block(h, qt))
                    items.append(lambda h=h: p1_final(h, sc))
                # outproj last: pure PE work hiding the pass-1 DMA-hop latency
                if sc >= 2:
                    for sub in range(4):
                        items.append(lambda sub=sub: outproj_sub(sc - 2, sub))
                return items

            class Filler:
                def __init__(self, items, slots):
                    self.items = items
                    self.per = len(items) / max(1, slots)
                    self.budget = 0.0
                    self.idx = 0

                def step(self, mult=1.0):
                    self.budget += self.per * mult
                    while self.idx < min(len(self.items), int(self.budget)):
                        self.items[self.idx]()
                        self.idx += 1

                def drain(self):
                    while self.idx < len(self.items):
                        self.items[self.idx]()
                        self.idx += 1

            # ================= stage 0 preamble =================
            for which, p in (("q", 0), ("q", 1), ("k", 0), ("k", 1)):
                for half in range(2):
                    proj_half(0, which, p, half)
            for h in HEAD_ORDER:
                kts_copy(0, h)
            for p in range(2):
                for half in range(2):
                    proj_half(0, "v", p, half)
            for kcl in range(4):
                vt_transpose(0, kcl)
            for h in HEAD_ORDER:
                for qt in range(4):
                    p1_block(h, qt)
                p1_final(h, 0)

            # ================= pipelined attention stages =================
            for sc in range(NQC):
                nkt = 4 * sc + 4
                # stage 3 keeps outproj(2) for the exp-throttled gaps of its
                # last head (emitted there directly) instead of generic filler
                fill = Filler(make_filler(sc + 1) if sc + 1 < NQC else [],
                              slots=HPG * (nkt + 1))
                for h in HEAD_ORDER:
                    # emit filler ahead of the first S2 so the PE queue has
                    # work while the S2 waits on cross-engine dependencies
                    fill.step()
                    pt_blks = {}
                    av = avps.tile([128, 512], dt.float32, name=f"ps_av_{h}_{sc}", tag="av")

                    def emit_av(kt, av=av, pt_blks=pt_blks, sc=sc, h=h, nkt=nkt):
                        # diag blocks: cols < 128r are fully masked (zero in
                        # pt), so skip them — earlier kt blocks own those q's
                        r = max(0, kt - 4 * sc)
                        nc.tensor.matmul(av[0:65, 128 * r:], vkd[:, kt, h, :],
                                         pt_blks[kt][:, 128 * r:],
                                         start=(kt == 0), stop=(kt == nkt - 1),
                                         skip_group_check=True)

                    for kt in range(nkt):
                        ps = psum.tile([128, 512], dt.float32, name=f"ps_s2_{h}_{sc}_{kt}", tag="ps")
                        r = max(0, kt - 4 * sc)
                        nc.tensor.matmul(ps[:, 128 * r:],
                                         kT[h][:, kt * 128:(kt + 1) * 128],
                                         qT[h][:, sc * SQ + 128 * r:(sc + 1) * SQ],
                                         start=True, stop=True)
                        pt = ptp.tile([128, 512], dt.bfloat16, name=f"pt{h}_{sc}_{kt}", tag="pt")
                        nc.scalar.activation(pt[:, 128 * r:], ps[:, 128 * r:], AF.Exp)
                        if kt - 4 * sc >= 0:
                            # zero above-diagonal of the exp'd diag sub-block
                            nc.gpsimd.affine_select(
                                out=pt[:, 128 * r:128 * r + 128],
                                in_=pt[:, 128 * r:128 * r + 128],
                                compare_op=ALU.is_ge, fill=0.0,
                                base=0, pattern=[[1, 128]], channel_multiplier=-1)
                        pt_blks[kt] = pt
                        if kt >= AV_LAG:
                            emit_av(kt - AV_LAG)
                        fill.step()
                        if sc == NQC - 1 and h == HEAD_ORDER[-1] and kt % 4 == 3:
                            outproj_sub(NQC - 2, kt // 4)
                    for kt in range(max(0, nkt - AV_LAG), nkt):
                        emit_av(kt)
                    # normalize: broadcast the Z row to 64 partitions, then a
                    # full-rate approximate reciprocal (51 ULP, plenty for the
                    # softmax denominator) — no DMA round trips in the chain
                    zsb = rcp.tile([65, 512], dt.float32, name=f"zsb{h}_{sc}", tag="zsb", bufs=4)
                    nc.vector.tensor_copy(zsb[:], av[0:65, :])
                    # one row DMA to move Z to partition 0 (partition_broadcast
                    # broadcasts partition 0; engines cannot shift partitions)
                    zrow = rcp.tile([1, 512], dt.float32, name=f"zrow{h}_{sc}", tag="zrow", bufs=4)
                    nc.sync.dma_start(zrow[:], zsb[64:65, :])
                    zbc = rcbp.tile([64, 512], dt.float32, name=f"zbc{h}_{sc}", tag="zbc")
                    nc.gpsimd.partition_broadcast(zbc[:], zrow[:])
                    rcb = rcbp.tile([64, 512], dt.float32, name=f"rcb{h}_{sc}", tag="rcb")
                    nc.vector.reciprocal_approx_fast(rcb[:], zbc[:])
                    eh = h // 2
                    last = (sc == NQC - 1 and h == HEAD_ORDER[-1])
                    if last:
                        # dummy matmuls bridge the PE over the Z-chain latency
                        # so the HAM clock gate stays at 2.4 GHz for the final
                        # output projection
                        wp2 = psum.tile([128, 512], dt.float32, name="warmps2", tag="ps")
                        for _ in range(24):
                            nc.tensor.matmul(wp2[:, 0:128], ident_b[:], ident_b[:],
                                             start=True, stop=True)
                    if h % 2 == 0:
                        if last:
                            # final head of the kernel: normalize in 128-col
                            # pieces, launching each outproj subtile as soon
                            # as its columns land, to shorten the drain tail
                            for st in range(4):
                                cols = slice(sc * SQ + st * 128, sc * SQ + (st + 1) * 128)
                                nc.vector.tensor_tensor(attnT[0:64, eh, cols],
                                                        zsb[0:64, st * 128:(st + 1) * 128],
                                                        rcb[:, st * 128:(st + 1) * 128],
                                                        ALU.mult)
                                outproj_sub(sc, st)
                        else:
                            nc.vector.tensor_tensor(attnT[0:64, eh, sc * SQ:(sc + 1) * SQ],
                                                    zsb[0:64, :], rcb[:], ALU.mult)
                    else:
                        att = ttp.tile([64, 512], dt.float16, name=f"att{h}_{sc}", tag="att")
                        nc.vector.tensor_tensor(att[:], zsb[0:64, :], rcb[:], ALU.mult)
                        nc.gpsimd.dma_start(attnT[64:128, eh, sc * SQ:(sc + 1) * SQ], att[:])
                fill.drain()
            nc.vector.tensor_copy(warms[:], wp2[0:1, 0:1])

    nc.compile()
    return nc


def _get_program():
    if "nc" not in _prog_cache:
        _prog_cache["nc"] = _build_program()
    return _prog_cache["nc"]


def _shard_inputs(residual, W_Q, W_K, W_V, W_O, b_Q, b_K, b_V, b_O):
    f32 = np.float32
    in_maps = []
    for core in range(N_CORES):
        b, g = core // G, core % G
        heads = list(range(HPG * g, HPG * g + HPG))
        # residual^T: [m, s] -> [mi, sc, mo, s-chunk] (per-stage contiguous)
        rT = np.ascontiguousarray(
            residual[b].T.reshape(MO, 128, NQC, SQ).transpose(1, 2, 0, 3)).astype(np.float16)

        def wstack(W, scale=1.0):
            # per pair p: [m, 128] -> [mi, mo, p, 128]
            pairs = []
            for p in range(2):
                wpair = np.concatenate([W[heads[2 * p]], W[heads[2 * p + 1]]], axis=1) * scale
                pairs.append(wpair.reshape(MO, 128, 128).transpose(1, 0, 2))
            return np.ascontiguousarray(np.stack(pairs, axis=2)).astype(np.float16)

        wq = wstack(W_Q, 0.125)
        wk = wstack(W_K)
        wv = wstack(W_V)
        bq = np.stack([np.concatenate([b_Q[heads[2 * p]], b_Q[heads[2 * p + 1]]]) * 0.125
                       for p in range(2)], axis=1).astype(f32)
        bk = np.stack([np.concatenate([b_K[heads[2 * p]], b_K[heads[2 * p + 1]]])
                       for p in range(2)], axis=1).astype(f32)
        wo = np.ascontiguousarray(
            W_O[256 * g:256 * (g + 1)].reshape(2, 128, D_MODEL).transpose(1, 0, 2)
        ).astype(np.float16)
        in_maps.append(dict(resT=rT, wq=wq, wk=wk, wv=wv, bq=np.ascontiguousarray(bq),
                            bk=np.ascontiguousarray(bk), wo=wo))
    return in_maps


def _run(inputs, trace=False):
    nc = _get_program()
    in_maps = _shard_inputs(**inputs)
    res = run_bass_kernel_spmd(nc, in_maps, core_ids=list(range(N_CORES)), trace=trace)
    # host-side reduce over the 4 head-group cores of each batch + folded bias
    W_O, b_V, b_O = inputs["W_O"], inputs["b_V"], inputs["b_O"]
    c = (b_O + sum(b_V[h] @ W_O[64 * h:64 * (h + 1)] for h in range(NUM_HEADS))
         ).astype(np.float32)
    out = np.empty((BATCH, D_SEQ, D_MODEL), np.float32)
    for b in range(BATCH):
        acc = res.results[b * G]["out_part"].astype(np.float32).copy()
        for g in range(1, G):
            acc += res.results[b * G + g]["out_part"]
        out[b] = acc + c[None, :]
    return out, res


def kernel(**inputs):
    out, _ = _run(inputs, trace=False)
    return out


# revision 58
# speedup vs baseline: 1.0578x; 1.0134x over previous
"""Trainium2 Bass kernel for causal multi-head attention (nn_Attention_3161095930536).

Model: batch=2, seq=2048, d_model=1024, 16 heads x 64. Reference computes
QKV projections + causal softmax attention + output projection (+ biases).

Sharding over 8 NeuronCores: core = (batch b = core//4) x (head-group
g = core%4, 4 heads each). Each core computes its head-group's attention and
a partial output projection into DRAM; the HOST sums the 4 partials per batch
(and adds the folded output bias), keeping the device critical path free of
collectives.

v3: single software-pipelined stream over the 4 seq chunks (sc). The v2
two-phase design (project everything, then attend) left the PE idle in
bursts during the early attention chunks, dropping the HAM clock gate to
K=4/8 (1.2 GHz) for ~90us of the kernel, and serialized ~30us of input DMA
at the start. Here the projections / pass-1 / V transposes for chunk sc+1
are emitted as *filler* inside the attention stream of chunk sc, so:
 - the PE has dense matmul work end to end (HAM stays at 2.4 GHz),
 - the projection-phase ACT work (bias adds) overlaps the exp stream,
 - the startup residual DMA overlaps stage-0 projections.

Other changes vs v2 (each validated against a perfetto trace):
 - exp runs on the UNMASKED diagonal scores (bounded: pass-1's row max is
   over legal keys, so masked entries exceed it by at most the score range
   ~O(10), well inside exp's fp32/bf16 range); the above-diagonal region of
   the bf16 P tile is then zeroed by gpsimd affine_select. This takes the
   DVE mask-add out of the S2->exp chain entirely.
 - input DMAs are emitted up front as FEW, LARGE transfers (the Sync engine
   pays ~0.6us of issue time per DMA instruction) split across both HWDGE
   queues (SP + Activation), with the residual laid out per-stage-contiguous
   on the host; latency-tolerant hops (odd-head qT/attnT stores, partial-out
   stores) go through the gpsimd SWDGE queue instead of Sync.
 - softmax normalize: one row-DMA moves Z to partition 0, then
   gpsimd partition_broadcast + DVE reciprocal_approx_fast (51-ULP, full
   rate) -- the v2 row->column->row DMA round trip (and its 2x ~2.6us
   SBUF-DMA completion latency) is gone. DVE's exact reciprocal is an
   8-cycle/element iterative op and must never touch a [1,512] row.
 - warmup: dummy ident matmuls + a dummy exp + a dummy gpsimd DMA/broadcast
   at t~0 pull the HAM clock-gate warmup, the ACT exp-table load (~2.7us)
   and the two Q7 IRAM library loads (~6us each) into the startup DMA
   shadow; another dummy-matmul bridge keeps the PE warm across the final
   head's normalize chain so the last output projection runs at 2.4 GHz.
 - the final chunk's output projection is emitted piecewise behind the last
   head's normalize (128 cols at a time), drains psum via ACT (idle after
   the last exp), and alternates its output DMAs across both HWDGE queues.

Carried over from v2: fp16 activations/weights (bf16 P/V for exp range),
fused -max row in the S2 matmul (row 64 of qT/kT), stride-16 sampled-key
pass-1 row max (underestimates only; softmax shift-invariance makes any
shift exact), V^T computed with stationary weights then PE-transposed into
[k, d] slabs, AV riding 5 blocks behind S2, host-side reduce of the 4
per-batch partials with b_V/b_O folded into one bias row.
"""

import numpy as np

import concourse.bass as bass
import concourse.mybir as mybir
import concourse.tile as tile
from concourse import bacc
from concourse.bass_utils import run_bass_kernel_spmd
from concourse.masks import make_identity

dt = mybir.dt
AF = mybir.ActivationFunctionType
ALU = mybir.AluOpType
AX = mybir.AxisListType

NUM_HEADS = 16
D_MODEL = 1024
D_HEAD = 64
D_SEQ = 2048
BATCH = 2
N_CORES = 8
HPG = 4          # heads per group (per core)
G = 4            # groups per batch
SQ = 512         # seq chunk (pipeline stage)
MO = D_MODEL // 128   # 8 m-chunks
NQT = D_SEQ // 128    # 16 q tiles
NQC = D_SEQ // SQ     # 4 seq chunks / stages
STRIDE = 16           # pass-1 past-key subsample stride
NSAMP = (D_SEQ - 128) // STRIDE   # 120 sampled past keys max
AV_LAG = 5            # A*V rides this many blocks behind S2
# last head of each stage is even -> its attnT write is a direct DVE store
# (no DMA down), shortening the chain into the next stage's outproj
HEAD_ORDER = [1, 3, 0, 2]

_prog_cache = {}


def _build_program():
    nc = bacc.Bacc("TRN2", target_bir_lowering=False, debug=False,
                   num_devices=N_CORES)

    resT_in = nc.dram_tensor("resT", [128, NQC, MO, SQ], dt.float16, kind="ExternalInput").ap()
    wq_in = nc.dram_tensor("wq", [128, MO, 2, 128], dt.float16, kind="ExternalInput").ap()
    wk_in = nc.dram_tensor("wk", [128, MO, 2, 128], dt.float16, kind="ExternalInput").ap()
    wv_in = nc.dram_tensor("wv", [128, MO, 2, 128], dt.float16, kind="ExternalInput").ap()
    bq_in = nc.dram_tensor("bq", [128, 2], dt.float32, kind="ExternalInput").ap()
    bk_in = nc.dram_tensor("bk", [128, 2], dt.float32, kind="ExternalInput").ap()
    wo_in = nc.dram_tensor("wo", [128, 2, D_MODEL], dt.float16, kind="ExternalInput").ap()
    out_io = nc.dram_tensor("out_part", [D_SEQ, D_MODEL], dt.float32, kind="ExternalOutput").ap()

    with tile.TileContext(nc) as tc:
        from contextlib import ExitStack
        outer = ExitStack()
        with outer:
            const = outer.enter_context(tc.tile_pool(name="const", bufs=1))
            qkp = outer.enter_context(tc.tile_pool(name="qkp", bufs=1))
            vp = outer.enter_context(tc.tile_pool(name="vp", bufs=1))
            statp = outer.enter_context(tc.tile_pool(name="statp", bufs=1))
            mxsp = outer.enter_context(tc.tile_pool(name="mxsp", bufs=2))
            scrp = outer.enter_context(tc.tile_pool(name="scrp", bufs=3))
            rcp = outer.enter_context(tc.tile_pool(name="rcp", bufs=4))
            rcbp = outer.enter_context(tc.tile_pool(name="rcbp", bufs=4))
            ttp = outer.enter_context(tc.tile_pool(name="ttp", bufs=4))
            ptp = outer.enter_context(tc.tile_pool(name="ptp", bufs=22))
            atp = outer.enter_context(tc.tile_pool(name="atp", bufs=1))
            osp = outer.enter_context(tc.tile_pool(name="osp", bufs=3))
            rp = outer.enter_context(tc.tile_pool(name="rp", bufs=1))
            wp = outer.enter_context(tc.tile_pool(name="wp", bufs=1))
            qtp = outer.enter_context(tc.tile_pool(name="qtp", bufs=3))
            vtp = outer.enter_context(tc.tile_pool(name="vtp", bufs=4))
            psum = outer.enter_context(tc.tile_pool(name="psum", bufs=3, space="PSUM"))
            projps = outer.enter_context(tc.tile_pool(name="projps", bufs=2, space="PSUM"))
            p1ps = outer.enter_context(tc.tile_pool(name="p1ps", bufs=2, space="PSUM"))
            avps = outer.enter_context(tc.tile_pool(name="avps", bufs=1, space="PSUM"))
            dram = outer.enter_context(tc.tile_pool(name="dram", bufs=1, space="DRAM"))

            # ---- constants ----
            # wdum first: a memset-only dummy weight so the PE warmup matmuls
            # don't wait for make_identity's gpsimd iota -> DVE cast chain
            wdum = const.tile([128, 128], dt.bfloat16, name="wdum")
            nc.vector.memset(wdum[:], 0.0)
            ident = const.tile([128, 128], dt.float32r, name="ident")
            ident_f = const.tile([128, 128], dt.float32, name="ident_f")
            make_identity(nc, ident_f[:])
            nc.vector.tensor_copy(ident[:], ident_f[:])
            ident_b = const.tile([128, 128], dt.bfloat16, name="ident_b")
            nc.vector.tensor_copy(ident_b[:], ident_f[:])

            # pass-1 diag mask (S [q, k] orientation: keep where j <= p)
            trimaskT = const.tile([128, 128 + NSAMP], dt.float32, name="trimaskT")
            nc.gpsimd.memset(trimaskT[:], 0.0)
            nc.gpsimd.affine_select(out=trimaskT[:, 0:128], in_=trimaskT[:, 0:128],
                                    compare_op=ALU.is_ge, fill=-1e30,
                                    base=0, pattern=[[-1, 128]], channel_multiplier=1)

            bqs = const.tile([128, 2], dt.float32, name="bqs")
            bks = const.tile([128, 2], dt.float32, name="bks")
            # issue the first gpsimd DMA and partition_broadcast immediately:
            # each triggers a one-time ~6us IRAM library load on the Q7 cores
            # that must overlap the startup DMA wait, not stall the gpsimd
            # queue mid-kernel
            nc.gpsimd.dma_start(bqs[:], bq_in[:])
            warmb = const.tile([64, 32], dt.float32, name="warmb")
            nc.gpsimd.partition_broadcast(warmb[:], ident_f[0:1, 0:32])
            nc.sync.dma_start(bks[:], bk_in[:])

            # dummy matmuls: keep the PE busy from ~1.5us so the HAM clock
            # gate reaches K=8/8 (2.4 GHz) before the first projection's
            # input DMA lands (~10us); otherwise the whole first projection
            # wave runs at 1.2 GHz
            wps = psum.tile([128, 512], dt.float32, name="warmps", tag="ps")
            for _ in range(36):
                nc.tensor.matmul(wps[:, 0:128], wdum[:], wdum[:],
                                 start=True, stop=True)
            warms = const.tile([1, 1], dt.float32, name="warms")
            nc.vector.tensor_copy(warms[:], wps[0:1, 0:1])
            # dummy exp: loads the ACT table set (~2.7us) during startup
            warme = const.tile([1, 1], dt.float32, name="warme")
            nc.scalar.activation(warme[:], wdum[0:1, 0:1], AF.Exp)


            # ---- persistent activations ----
            qT = [qkp.tile([65, D_SEQ], dt.float16, name=f"qT{h}") for h in range(HPG)]
            kT = [qkp.tile([65, D_SEQ], dt.float16, name=f"kT{h}") for h in range(HPG)]
            kTs = [qkp.tile([64, NSAMP], dt.float16, name=f"kTs{h}") for h in range(HPG)]
            # V in [k, d] layout, one 65-wide slab per head: cols 0:64 = V_h, col 64 = 1.0
            vkd = vp.tile([128, NQT, HPG, 65], dt.bfloat16, name="vkd")

            attnT = atp.tile([128, 2, D_SEQ], dt.float16, name="attnT")
            wo = atp.tile([128, 2, D_MODEL], dt.float16, name="wo")

            # negmx[h][sc] col qt%4 = -(max over sampled+diag keys) per q row
            negmx = [[statp.tile([128, 32], dt.float32r, name=f"negmx{h}_{sc}")
                      for sc in range(NQC)] for h in range(HPG)]

            # ---- weights + residual: few big DMAs (the Sync engine spends
            # ~0.6us of issue time per DMA instruction, so DMA count matters
            # more than transfer size); residual is laid out per-stage
            # contiguous on the host so each stage is one descriptor run ----
            wq = wp.tile([128, MO, 2, 128], dt.float16, name="wq")
            wk = wp.tile([128, MO, 2, 128], dt.float16, name="wk")
            wv = wp.tile([128, MO, 2, 128], dt.float16, name="wv")
            resT = [rp.tile([128, MO, SQ], dt.float16, name=f"resT{sc}")
                    for sc in range(NQC)]
            # inputs split across BOTH hardware DGE queues (SP + Activation)
            # so the ~6.5MB load halves in time; the ACT engine only pays a
            # ~0.6us issue cost per DMA, long before its first real work
            nc.sync.dma_start(wq[:, 0:4], wq_in[:, 0:4])
            nc.scalar.dma_start(wk[:, 0:4], wk_in[:, 0:4])
            nc.sync.dma_start(resT[0][:, 0:4], resT_in[:, 0, 0:4])
            nc.scalar.dma_start(wk[:, 4:8], wk_in[:, 4:8])
            nc.sync.dma_start(wq[:, 4:8], wq_in[:, 4:8])
            nc.sync.dma_start(resT[0][:, 4:8], resT_in[:, 0, 4:8])
            nc.scalar.dma_start(wv[:], wv_in[:])
            nc.scalar.dma_start(wo[:], wo_in[:])
            nc.sync.dma_start(resT[1][:], resT_in[:, 1])
            nc.scalar.dma_start(resT[2][:], resT_in[:, 2])
            nc.sync.dma_start(resT[3][:], resT_in[:, 3])
            # persistent-tile memsets AFTER the warmup emission so the DVE
            # queue doesn't delay the ident casts the warmup matmuls need
            nc.vector.memset(vkd[:, :, :, 64], 1.0)
            for h in range(HPG):
                nc.vector.memset(kT[h][64:65, :], 1.0)

            # ================= stage building blocks =================

            def proj_half(sc, which, p, half):
                """Half a projection group: 4 accumulating matmuls; the
                second half also drains psum -> qT/kT/vT."""
                w, dst, bias = {
                    "q": (wq, qT, bqs), "k": (wk, kT, bks), "v": (wv, None, None),
                }[which]
                if half == 0:
                    ps = projps.tile([128, SQ], dt.float32,
                                     name=f"ps_{which}{p}_{sc}", tag="proj")
                    proj_half.live[(sc, which, p)] = ps
                else:
                    ps = proj_half.live.pop((sc, which, p))
                for mo in range(4 * half, 4 * half + 4):
                    nc.tensor.matmul(ps[:], w[:, mo, p, :], resT[sc][:, mo],
                                     start=(mo == 0), stop=(mo == MO - 1))
                if half == 0:
                    return
                cols = slice(sc * SQ, (sc + 1) * SQ)
                if which == "v":
                    vt = vtp.tile([128, SQ], dt.bfloat16, name=f"vT{p}_{sc}", tag="vt")
                    nc.scalar.copy(vt[:], ps[:])
                    proj_half.vt[(sc, p)] = vt
                    return
                # even head: direct ACT with bias
                nc.scalar.activation(dst[2 * p][0:64, cols], ps[0:64, :], AF.Identity,
                                     bias=bias[0:64, p:p + 1], scale=1.0)
                # odd head: aligned ACT into tmp rows 64:128, then DMA down
                qt_t = qtp.tile([128, SQ], dt.float16, name=f"qtmp_{which}{p}{sc}", tag="qtmp")
                nc.scalar.activation(qt_t[64:128, :], ps[64:128, :], AF.Identity,
                                     bias=bias[64:128, p:p + 1], scale=1.0)
                nc.gpsimd.dma_start(dst[2 * p + 1][0:64, cols], qt_t[64:128, :])
            proj_half.live = {}
            proj_half.vt = {}

            def kts_copy(sc, h):
                # sampled keys (stride 16) newly available from seq chunk sc
                lo = sc * SQ // STRIDE
                hi = min((sc + 1) * SQ, D_SEQ - 128) // STRIDE
                if hi <= lo:
                    return
                nc.vector.tensor_copy(
                    kTs[h][:, lo:hi],
                    kT[h][0:64, sc * SQ:hi * STRIDE:STRIDE])

            def vt_transpose(sc, kc_local):
                kc = 4 * sc + kc_local
                ps = psum.tile([128, 512], dt.float32, name=f"ps_v{kc}", tag="ps")
                pb = ps[:].bitcast(dt.bfloat16)
                for c in range(2):
                    nc.tensor.transpose(pb[:, 512 * c:512 * c + 128],
                                        proj_half.vt[(sc, c)][:, kc_local * 128:(kc_local + 1) * 128],
                                        ident_b[:])
                nc.vector.tensor_copy(
                    vkd[:, kc, :, 0:64].rearrange("p (c h) d -> p c h d", c=2),
                    pb.rearrange("p (c r) -> p c r", c=2)[:, :, 0:128]
                      .rearrange("p c (h d) -> p c h d", h=2))

            def p1_block(h, qt):
                nsamp = (qt * 128) // STRIDE
                ncols = 128 + nsamp
                ps = p1ps.tile([128, 512], dt.float32, name=f"ps_p1_{h}_{qt}", tag="p1")
                qstat = qT[h][0:64, qt * 128:(qt + 1) * 128]
                nc.tensor.matmul(ps[:, 0:128], qstat,
                                 kT[h][0:64, qt * 128:(qt + 1) * 128],
                                 start=True, stop=True)
                if nsamp:
                    nc.tensor.matmul(ps[:, 128:ncols], qstat,
                                     kTs[h][:, 0:nsamp], start=True, stop=True)
                scr = scrp.tile([128, 128 + NSAMP], dt.bfloat16,
                                name=f"scr{h}_{qt}", tag="scr")
                nc.vector.tensor_tensor(scr[:, 0:ncols], ps[:, 0:ncols],
                                        trimaskT[:, 0:ncols], ALU.add)
                nc.vector.tensor_reduce(negmx[h][qt // 4][:, (qt % 4):(qt % 4) + 1],
                                        scr[:, 0:ncols], AX.X, ALU.max, negate=True)

            def p1_final(h, sc):
                # -max column -> row 64 of qT[h] for this chunk (via transpose
                # on the PE and a DRAM partition->row reshape hop)
                pst = psum.tile([128, 512], dt.float32r, name=f"ps_tp{h}_{sc}", tag="ps")
                nc.tensor.transpose(pst[0:32, 0:128], negmx[h][sc][:], ident[:])
                mxs = mxsp.tile([4, 128], dt.float16, name=f"mxs{h}_{sc}", tag="mxs")
                nc.vector.tensor_copy(mxs[:], pst[0:4, 0:128].bitcast(dt.float32))
                mrow = dram.tile([4, 128], dt.float16, name=f"mrow{h}_{sc}")
                nc.sync.dma_start(mrow[:], mxs[:])
                nc.sync.dma_start(qT[h][64:65, sc * SQ:(sc + 1) * SQ],
                                  mrow[:].rearrange("t f -> (t f)").unsqueeze(0))

            def outproj_sub(qc, sub):
                st = 4 * qc + sub
                final = (qc == NQC - 1)
                osb = osp.tile([128, D_MODEL], dt.float32, name=f"osb{st}", tag="osb")
                for mc in range(2):
                    ps = psum.tile([128, 512], dt.float32, name=f"ps_o{st}_{mc}", tag="ps")
                    for eo in range(2):
                        nc.tensor.matmul(ps[:], attnT[:, eo, st * 128:(st + 1) * 128],
                                         wo[:, eo, mc * 512:(mc + 1) * 512],
                                         start=(eo == 0), stop=(eo == 1))
                    # the final chunk drains via ACT (idle after the last exp)
                    # so its psum copies don't serialize with the matmuls;
                    # early chunks (running in stages 1-2 where ACT has slack)
                    # split their two drains across ACT+DVE. outproj(2) runs
                    # inside stage 3's exp-saturated last head: all-DVE.
                    if final or (qc <= 1 and mc == 0):
                        nc.scalar.copy(osb[:, mc * 512:(mc + 1) * 512], ps[:])
                    else:
                        nc.vector.tensor_copy(osb[:, mc * 512:(mc + 1) * 512], ps[:])
                # final-chunk output alternates the two HWDGE queues so the
                # last transfers run in parallel instead of serializing
                eng = (nc.sync if sub % 2 == 0 else nc.scalar) if final else nc.gpsimd
                eng.dma_start(out_io[st * 128:(st + 1) * 128, :], osb[:])

            # ---- filler machinery: work for stage sc+1, emitted inside
            # the attention stream of stage sc ----
            def make_filler(sc):
                """Filler items that prepare stage sc (emitted during stage
                sc-1's attention). Pass-1 comes right after the q/k
                projections so its DMA-hop latency hides behind the V
                projections / transposes; outproj has no dependencies on this
                stage and goes last as pure PE filler."""
                items = []
                for which, p in (("q", 0), ("q", 1), ("k", 0), ("k", 1)):
                    for half in range(2):
                        items.append(lambda w=which, p=p, hf=half: proj_half(sc, w, p, hf))
                for h in HEAD_ORDER:
                    items.append(lambda h=h: kts_copy(sc, h))
                # V before pass-1: its DVE psum drain must not queue behind
                # pass-1's DVE adds/reduces (the V transposes need it)
                for p in range(2):
                    for half in range(2):
                        items.append(lambda p=p, hf=half: proj_half(sc, "v", p, hf))
                for kcl in range(4):
                    items.append(lambda kcl=kcl: vt_transpose(sc, kcl))
                for h in HEAD_ORDER:
                    for qt in range(4 * sc, 4 * sc + 4):
                        items.append(lambda h=h, qt=qt: p1_block(h, qt))
                    items.append(lambda h=h: p1_final(h, sc))
                # outproj last: pure PE work hiding the pass-1 DMA-hop latency
                if sc >= 2:
                    for sub in range(4):
                        items.append(lambda sub=sub: outproj_sub(sc - 2, sub))
                return items

            class Filler:
                def __init__(self, items, slots):
                    self.items = items
                    self.per = len(items) / max(1, slots)
                    self.budget = 0.0
                    self.idx = 0

                def step(self, mult=1.0):
                    self.budget += self.per * mult
                    while self.idx < min(len(self.items), int(self.budget)):
                        self.items[self.idx]()
                        self.idx += 1

                def drain(self):
                    while self.idx < len(self.items):
                        self.items[self.idx]()
                        self.idx += 1

            # ================= stage 0 preamble =================
            for which, p in (("q", 0), ("q", 1), ("k", 0), ("k", 1)):
                for half in range(2):
                    proj_half(0, which, p, half)
            for h in HEAD_ORDER:
                kts_copy(0, h)
            for p in range(2):
                for half in range(2):
                    proj_half(0, "v", p, half)
            for kcl in range(4):
                vt_transpose(0, kcl)
            for h in HEAD_ORDER:
                for qt in range(4):
                    p1_block(h, qt)
                p1_final(h, 0)

            # ================= pipelined attention stages =================
            for sc in range(NQC):
                nkt = 4 * sc + 4
                # stage 3 keeps outproj(2) for the exp-throttled gaps of its
                # last head (emitted there directly) instead of generic filler
                fill = Filler(make_filler(sc + 1) if sc + 1 < NQC else [],
                              slots=HPG * (nkt + 1))
                for h in HEAD_ORDER:
                    # emit filler ahead of the first S2 so the PE queue has
                    # work while the S2 waits on cross-engine dependencies
                    fill.step()
                    pt_blks = {}
                    av = avps.tile([128, 512], dt.float32, name=f"ps_av_{h}_{sc}", tag="av")

                    def emit_av(kt, av=av, pt_blks=pt_blks, sc=sc, h=h, nkt=nkt):
                        # diag blocks: cols < 128r are fully masked (zero in
                        # pt), so skip them — earlier kt blocks own those q's
                        r = max(0, kt - 4 * sc)
                        nc.tensor.matmul(av[0:65, 128 * r:], vkd[:, kt, h, :],
                                         pt_blks[kt][:, 128 * r:],
                                         start=(kt == 0), stop=(kt == nkt - 1),
                                         skip_group_check=True)

                    for kt in range(nkt):
                        ps = psum.tile([128, 512], dt.float32, name=f"ps_s2_{h}_{sc}_{kt}", tag="ps")
                        r = max(0, kt - 4 * sc)
                        nc.tensor.matmul(ps[:, 128 * r:],
                                         kT[h][:, kt * 128:(kt + 1) * 128],
                                         qT[h][:, sc * SQ + 128 * r:(sc + 1) * SQ],
                                         start=True, stop=True)
                        pt = ptp.tile([128, 512], dt.bfloat16, name=f"pt{h}_{sc}_{kt}", tag="pt")
                        nc.scalar.activation(pt[:, 128 * r:], ps[:, 128 * r:], AF.Exp)
                        if kt - 4 * sc >= 0:
                            # zero above-diagonal of the exp'd diag sub-block
                            nc.gpsimd.affine_select(
                                out=pt[:, 128 * r:128 * r + 128],
                                in_=pt[:, 128 * r:128 * r + 128],
                                compare_op=ALU.is_ge, fill=0.0,
                                base=0, pattern=[[1, 128]], channel_multiplier=-1)
                        pt_blks[kt] = pt
                        if kt >= AV_LAG:
                            emit_av(kt - AV_LAG)
                        fill.step()
                        if sc == NQC - 1 and h == HEAD_ORDER[-1] and kt % 4 == 3:
                            outproj_sub(NQC - 2, kt // 4)
                    for kt in range(max(0, nkt - AV_LAG), nkt):
                        emit_av(kt)
                    # normalize: broadcast the Z row to 64 partitions, then a
                    # full-rate approximate reciprocal (51 ULP, plenty for the
                    # softmax denominator) — no DMA round trips in the chain
                    zsb = rcp.tile([65, 512], dt.float32, name=f"zsb{h}_{sc}", tag="zsb", bufs=4)
                    nc.vector.tensor_copy(zsb[:], av[0:65, :])
                    # one row DMA to move Z to partition 0 (partition_broadcast
                    # broadcasts partition 0; engines cannot shift partitions)
                    zrow = rcp.tile([1, 512], dt.float32, name=f"zrow{h}_{sc}", tag="zrow", bufs=4)
                    nc.sync.dma_start(zrow[:], zsb[64:65, :])
                    zbc = rcbp.tile([64, 512], dt.float32, name=f"zbc{h}_{sc}", tag="zbc")
                    nc.gpsimd.partition_broadcast(zbc[:], zrow[:])
                    rcb = rcbp.tile([64, 512], dt.float32, name=f"rcb{h}_{sc}", tag="rcb")
                    nc.vector.reciprocal_approx_fast(rcb[:], zbc[:])
                    eh = h // 2
                    last = (sc == NQC - 1 and h == HEAD_ORDER[-1])
                    if last:
                        # dummy matmuls bridge the PE over the Z-chain latency
                        # so the HAM clock gate stays at 2.4 GHz for the final
                        # output projection
                        wp2 = psum.tile([128, 512], dt.float32, name="warmps2", tag="ps")
                        for _ in range(24):
                            nc.tensor.matmul(wp2[:, 0:128], wdum[:], wdum[:],
                                             start=True, stop=True)
                    if h % 2 == 0:
                        if last:
                            # final head of the kernel: normalize in 128-col
                            # pieces, launching each outproj subtile as soon
                            # as its columns land, to shorten the drain tail
                            for st in range(4):
                                cols = slice(sc * SQ + st * 128, sc * SQ + (st + 1) * 128)
                                nc.vector.tensor_tensor(attnT[0:64, eh, cols],
                                                        zsb[0:64, st * 128:(st + 1) * 128],
                                                        rcb[:, st * 128:(st + 1) * 128],
                                                        ALU.mult)
                                outproj_sub(sc, st)
                        else:
                            nc.vector.tensor_tensor(attnT[0:64, eh, sc * SQ:(sc + 1) * SQ],
                                                    zsb[0:64, :], rcb[:], ALU.mult)
                    else:
                        att = ttp.tile([64, 512], dt.float16, name=f"att{h}_{sc}", tag="att")
                        nc.vector.tensor_tensor(att[:], zsb[0:64, :], rcb[:], ALU.mult)
                        nc.gpsimd.dma_start(attnT[64:128, eh, sc * SQ:(sc + 1) * SQ], att[:])
                fill.drain()
            nc.vector.tensor_copy(warms[:], wp2[0:1, 0:1])

    nc.compile()
    return nc


def _get_program():
    if "nc" not in _prog_cache:
        _prog_cache["nc"] = _build_program()
    return _prog_cache["nc"]


def _shard_inputs(residual, W_Q, W_K, W_V, W_O, b_Q, b_K, b_V, b_O):
    f32 = np.float32
    in_maps = []
    for core in range(N_CORES):
        b, g = core // G, core % G
        heads = list(range(HPG * g, HPG * g + HPG))
        # residual^T: [m, s] -> [mi, sc, mo, s-chunk] (per-stage contiguous)
        rT = np.ascontiguousarray(
            residual[b].T.reshape(MO, 128, NQC, SQ).transpose(1, 2, 0, 3)).astype(np.float16)

        def wstack(W, scale=1.0):
            # per pair p: [m, 128] -> [mi, mo, p, 128]
            pairs = []
            for p in range(2):
                wpair = np.concatenate([W[heads[2 * p]], W[heads[2 * p + 1]]], axis=1) * scale
                pairs.append(wpair.reshape(MO, 128, 128).transpose(1, 0, 2))
            return np.ascontiguousarray(np.stack(pairs, axis=2)).astype(np.float16)

        wq = wstack(W_Q, 0.125)
        wk = wstack(W_K)
        wv = wstack(W_V)
        bq = np.stack([np.concatenate([b_Q[heads[2 * p]], b_Q[heads[2 * p + 1]]]) * 0.125
                       for p in range(2)], axis=1).astype(f32)
        bk = np.stack([np.concatenate([b_K[heads[2 * p]], b_K[heads[2 * p + 1]]])
                       for p in range(2)], axis=1).astype(f32)
        wo = np.ascontiguousarray(
            W_O[256 * g:256 * (g + 1)].reshape(2, 128, D_MODEL).transpose(1, 0, 2)
        ).astype(np.float16)
        in_maps.append(dict(resT=rT, wq=wq, wk=wk, wv=wv, bq=np.ascontiguousarray(bq),
                            bk=np.ascontiguousarray(bk), wo=wo))
    return in_maps


def _run(inputs, trace=False):
    nc = _get_program()
    in_maps = _shard_inputs(**inputs)
    res = run_bass_kernel_spmd(nc, in_maps, core_ids=list(range(N_CORES)), trace=trace)
    # host-side reduce over the 4 head-group cores of each batch + folded bias
    W_O, b_V, b_O = inputs["W_O"], inputs["b_V"], inputs["b_O"]
    c = (b_O + sum(b_V[h] @ W_O[64 * h:64 * (h + 1)] for h in range(NUM_HEADS))
         ).astype(np.float32)
    out = np.empty((BATCH, D_SEQ, D_MODEL), np.float32)
    for b in range(BATCH):
        acc = res.results[b * G]["out_part"].astype(np.float32).copy()
        for g in range(1, G):
            acc += res.results[b * G + g]["out_part"]
        out[b] = acc + c[None, :]
    return out, res


def kernel(**inputs):
    out, _ = _run(inputs, trace=False)
    return out
